# revision 21
# baseline (speedup 1.0000x reference)
"""Trainium2 Bass kernel for nn_BaselineAttnDecoder.

Data-parallel over 8 NeuronCores: each core handles 160 decode rows
(= 16 images x 10 rounds). All weights replicated. Per core:
  - question-encoder GRU (20 sequential steps, batch 160)
  - attention decoder GRU (21 sequential steps) with question + image
    attention, argmax re-embedding at step 19 (bf16 top-8 logits with
    exact f32 rescoring).

Layout: batch-stationary matmuls (lhsT = transposed activations
[feat, b], moving rhs = weight columns, N<=512) in bf16 with f32 PSUM
accumulation; gate math in f32; biases folded via augmented ones-rows,
rank-1 matmuls, and softmax-sum identities. Embedding rows gathered
just-in-time per step via indirect DMA and transposed on the PE.
"""
import numpy as np
import ml_dtypes

import concourse.bass as bass
import concourse.bacc as bacc
import concourse.mybir as mybir
import concourse.tile as tile
from concourse.masks import make_identity

F32 = mybir.dt.float32
BF16 = mybir.dt.float16  # 16-bit compute dtype (f16: 10-bit mantissa)
U32 = mybir.dt.uint32
AF = mybir.ActivationFunctionType
ALU = mybir.AluOpType
AX = mybir.AxisListType

D, H, V, K = 300, 512, 8835, 50
L, MAX_LEN, ROUNDS = 20, 21, 10
BS = 160
NCORES = 8
PBS = [128, 32]
BOFF = [0, 128]
IL = 256
VP = 18 * 512
NEG = -1.0e30


def bcast_mid(ap, reps):
    return bass.AP(tensor=ap.tensor, offset=ap.offset,
                   ap=[ap.ap[0], [0, reps], ap.ap[1]])


def bcast_in(ap, reps):
    return bass.AP(tensor=ap.tensor, offset=ap.offset,
                   ap=[ap.ap[0], ap.ap[1], [0, reps]])


class ColView:
    def __init__(self, base, col0):
        self.base, self.col0 = base, col0

    def __getitem__(self, key):
        rows, kt, sl = key
        return self.base[rows, kt, self.col0 + sl.start:self.col0 + sl.stop]


def build_nc():
    nc = bacc.Bacc()

    def din(name, shape, dt):
        return nc.dram_tensor(name, shape, dt, kind="ExternalInput")

    w_gi = din("w_gi", [128, 11, 3 * H], BF16)
    w_gh = din("w_gh", [128, 4, 3 * H], BF16)
    bhh_n = din("bhh_n", [1, H], BF16)
    w_egi = din("w_egi", [128, 3, 3 * H], BF16)
    w_egh = din("w_egh", [128, 4, 3 * H], BF16)
    ebhh_n = din("ebhh_n", [1, H], BF16)
    w_out = din("w_out", [128, 12, D], BF16)
    outb = din("outb", [1, D], BF16)
    w_qk = din("w_qk", [128, 4, K], BF16)
    qkb = din("qkb", [1, K], BF16)
    w_qv = din("w_qv", [128, 4, H], BF16)
    w_ak = din("w_ak", [128, 4, K], BF16)
    akb = din("akb", [1, K], BF16)
    w_ik = din("w_ik", [128, 2, K], BF16)
    ikb_c = din("ikb_c", [128, 1], F32)
    w_iv = din("w_iv", [128, 2, H], BF16)
    ivb_c = din("ivb_c", [128, 4], F32)
    qvb_c = din("qvb_c", [128, 4], F32)
    img_t = din("img_t", [128, 2, IL], BF16)
    emb_bf = din("emb_bf", [V, D], BF16)
    emb_aug = din("emb_aug", [V, D + 1], F32)
    embt_bf = din("embt_bf", [128, 3, VP], BF16)
    q_idx = din("q_idx", [128, 2 * L], U32)
    a_idx = din("a_idx", [128, 2 * L], U32)
    qe_mask = din("qe_mask", [128, 2, L], F32)
    ie_mask = din("ie_mask", [128, 2, IL], F32)

    out_o = nc.dram_tensor("out_o", [MAX_LEN, BS, D], F32, kind="ExternalOutput")

    with tile.TileContext(nc) as tc:
        with (
            tc.tile_pool(name="cw", bufs=1) as cw,
            tc.tile_pool(name="pers", bufs=1) as pers,
            tc.tile_pool(name="wk", bufs=2) as wk,
            tc.tile_pool(name="st", bufs=2) as st,
            tc.tile_pool(name="psg", bufs=5, space="PSUM") as psg,
            tc.tile_pool(name="pss", bufs=2, space="PSUM") as pss,
            tc.tile_pool(name="pst", bufs=1, space="PSUM") as pst,
        ):
            def load(pool, t, dt):
                s = pool.tile(list(t.shape), dt, name=t.name + "_sb")
                nc.sync.dma_start(s[:], t[:])
                return s

            s_qk = load(cw, w_qk, BF16)
            s_qv = load(cw, w_qv, BF16)
            s_ak = load(cw, w_ak, BF16)
            s_ik = load(cw, w_ik, BF16)
            s_iv = load(cw, w_iv, BF16)
            s_imgt = load(cw, img_t, BF16)
            s_bhh = load(cw, bhh_n, BF16)
            s_ebhh = load(cw, ebhh_n, BF16)
            s_outb = load(cw, outb, BF16)
            s_qkb = load(cw, qkb, BF16)
            s_akb = load(cw, akb, BF16)
            s_ikb = load(cw, ikb_c, F32)
            s_ivb = load(cw, ivb_c, F32)
            s_qvb = load(cw, qvb_c, F32)
            s_qidx = load(cw, q_idx, U32)
            s_aidx = load(cw, a_idx, U32)
            s_qem = load(cw, qe_mask, F32)
            s_iem = load(cw, ie_mask, F32)

            ident_bf = cw.tile([128, 128], BF16)
            make_identity(nc, ident_bf[:])
            ones_bf = cw.tile([1, 128], BF16)
            nc.vector.memset(ones_bf[:], 1.0)
            sid4 = cw.tile([128, 32], BF16)
            for g4 in range(4):
                nc.vector.tensor_copy(sid4[32 * g4:32 * (g4 + 1), :],
                                      ident_bf[0:32, 0:32])
            iota8 = cw.tile([128, 8], F32)
            nc.gpsimd.iota(iota8[:], pattern=[[1, 8]], base=0, channel_multiplier=0,
                           allow_small_or_imprecise_dtypes=True)

            hT = pers.tile([128, 4, BS], BF16)
            h_f = pers.tile([128, 2, H], F32)
            qk_b0 = pers.tile([128, L, K], BF16)
            qk_b1 = pers.tile([128, L, K], BF16)
            qv_b0 = pers.tile([128, L, H], BF16)
            qv_p1 = pers.tile([128, 5, H], BF16)
            qkbs = [qk_b0, qk_b1]
            ivv = pers.tile([128, 2, H], BF16)
            ikt = pers.tile([128, IL], BF16)
            qcT = pers.tile([128, 4, BS], BF16)
            icT = pers.tile([128, 4, BS], BF16)
            dec20 = pers.tile([128, 3, BS], BF16)

            nc.vector.memset(hT[:], 0.0)
            nc.vector.memset(h_f[:], 0.0)
            nc.vector.memset(dec20[32:64, 2, :], 0.0)
            nc.vector.memset(dec20[64:65, 2, :], 1.0)

            trctr = [0]

            def trslot():
                pt = pst.tile([128, 2, 128], BF16, tag="trp", name="pt")
                s = trctr[0] & 1
                trctr[0] += 1
                return pt[:, s, :]

            def tr(dst_sb_ap, src_sb_ap, pb, w, eng=None):
                pt = trslot()
                nc.tensor.transpose(pt[:w, :pb], src_sb_ap, ident_bf[:pb, :pb])
                (eng or nc.vector).tensor_copy(dst_sb_ap, pt[:w, :pb])

            def tr_add(dst_sb_ap, src_sb_ap, bias_ap, pb, w):
                pt = trslot()
                nc.tensor.transpose(pt[:w, :pb], src_sb_ap, ident_bf[:pb, :pb])
                nc.vector.tensor_scalar_add(dst_sb_ap, pt[:w, :pb], bias_ap)

            def fetch_x(idx_sb, t):
                xt = wk.tile([128, 3, BS], BF16, tag="xt", bufs=6, name="xt")
                nc.vector.memset(xt[32:64, 2, :], 0.0)
                nc.vector.memset(xt[64:65, 2, :], 1.0)
                for c, (pb, off) in enumerate(zip(PBS, BOFF)):
                    g = wk.tile([128, D], BF16, tag="gath", bufs=6, name="g")
                    nc.gpsimd.indirect_dma_start(
                        out=g[:pb], out_offset=None, in_=emb_bf[:],
                        in_offset=bass.IndirectOffsetOnAxis(
                            ap=idx_sb[:pb, 2 * t + c:2 * t + c + 1], axis=0))
                    for kt in range(3):
                        w = 128 if kt < 2 else D - 256
                        tr(xt[:w, kt, off:off + pb], g[:pb, kt * 128:kt * 128 + w],
                           pb, w)
                return xt

            def emit_group(ps_ap, pairs):
                n = len(pairs)
                for i, (lh, rh) in enumerate(pairs):
                    nc.tensor.matmul(ps_ap, lh, rh, start=(i == 0), stop=(i == n - 1))

            def gru_bt(bt, xsrc, xrows, w_rhs, gh_rhs, bhh_row, extra,
                       tform=False):
                pb, off = PBS[bt], BOFF[bt]
                sl = slice(off, off + pb)
                gps = []
                for ci in range(3):
                    cs = slice(ci * H, (ci + 1) * H)
                    ps = psg.tile([128, H], F32, tag="g", name="ps")
                    pairs = [(xsrc[slice(0, nr), kt, sl], w_rhs[:nr, kt, cs])
                             for kt, nr in enumerate(xrows)]
                    pairs += [(lt[:, lk, sl], w_rhs[:, rk, cs])
                              for (lt, lk, rk) in extra]
                    if ci < 2:
                        pairs += [(hT[:, kt, sl], gh_rhs[:, kt, cs])
                                  for kt in range(4)]
                    emit_group(ps[:pb, :], pairs)
                    gps.append(ps)
                ps_bn = psg.tile([128, H], F32, tag="g", name="ps_bn")
                pairs = [(hT[:, kt, sl], gh_rhs[:, kt, 2 * H:3 * H])
                         for kt in range(4)]
                pairs.append((ones_bf[:, :pb], bhh_row[:]))
                emit_group(ps_bn[:pb, :], pairs)

                ps_r, ps_z, ps_an = gps
                r = st.tile([128, H], F32, tag="r", bufs=1, name="r")
                z = st.tile([128, H], F32, tag="z", bufs=1, name="z")
                n = st.tile([128, H], F32, tag="n", bufs=1, name="n")
                t1 = st.tile([128, H], F32, tag="t1", bufs=1, name="t1")
                if tform:
                    # sigma(x) = (1 + tanh(x/2))/2 with 0.5/2.0 host-prescaled
                    # weights: keeps the whole decoder on the exp table set.
                    nc.scalar.activation(r[:pb], ps_r[:pb], AF.Tanh)
                    nc.scalar.activation(z[:pb], ps_z[:pb], AF.Tanh)
                    u = st.tile([128, H], F32, tag="u", bufs=1, name="u")
                    nc.gpsimd.tensor_scalar_add(u[:pb], r[:pb], 1.0)
                    nc.vector.tensor_mul(t1[:pb], u[:pb], ps_bn[:pb])
                    nc.vector.tensor_add(t1[:pb], t1[:pb], ps_an[:pb])
                    nc.scalar.activation(n[:pb], t1[:pb], AF.Tanh, scale=0.5)
                    zz = st.tile([128, H], F32, tag="zz", bufs=1, name="zz")
                    nc.vector.tensor_scalar(out=zz[:pb], in0=z[:pb],
                                            scalar1=0.5, scalar2=0.5,
                                            op0=ALU.mult, op1=ALU.add)
                    z = zz
                else:
                    nc.scalar.activation(r[:pb], ps_r[:pb], AF.Sigmoid)
                    nc.scalar.activation(z[:pb], ps_z[:pb], AF.Sigmoid)
                    nc.vector.tensor_mul(t1[:pb], r[:pb], ps_bn[:pb])
                    nc.vector.tensor_add(t1[:pb], t1[:pb], ps_an[:pb])
                    nc.scalar.activation(n[:pb], t1[:pb], AF.Tanh)
                d = st.tile([128, H], F32, tag="d", bufs=1, name="d")
                nc.gpsimd.tensor_sub(d[:pb], h_f[:pb, bt, :], n[:pb])
                nc.gpsimd.tensor_mul(d[:pb], d[:pb], z[:pb])
                nc.gpsimd.tensor_add(h_f[:pb, bt, :], d[:pb], n[:pb])
                hb = st.tile([128, H], BF16, tag="hb", bufs=1, name="hb")
                nc.scalar.copy(hb[:pb], h_f[:pb, bt, :])
                for kt in range(4):
                    tr(hT[:, kt, off:off + pb],
                       hb[:pb, kt * 128:(kt + 1) * 128], pb, 128)

            # ---------- image projections ----------
            for mt in range(2):
                psv = psg.tile([128, H], F32, tag="g", name="psv")
                emit_group(psv[:], [(s_imgt[:, kt, mt * 128:(mt + 1) * 128],
                                     s_iv[:, kt, :]) for kt in range(2)])
                nc.scalar.copy(ivv[:, mt, :], psv[:])
            psik = pss.tile([128, IL], F32, tag="s", name="psik")
            emit_group(psik[:K, :], [(s_ik[:, kt, :], s_imgt[:, kt, :])
                                     for kt in range(2)])
            nc.vector.tensor_scalar_add(ikt[:K, :], psik[:K, :], s_ikb[:K, :])

            # ---------- encoder ----------
            with tc.tile_pool(name="qp", bufs=1) as qp:
                s_egi = load(qp, w_egi, BF16)
                s_egh = load(qp, w_egh, BF16)
                for t in range(L):
                    xt = fetch_x(s_qidx, t)
                    for bt in range(2):
                        gru_bt(bt, ColView(xt, 0), [128, 128, 65],
                               s_egi, s_egh, s_ebhh, [])
                    for bt in range(2):
                        pb, off = PBS[bt], BOFF[bt]
                        sl = slice(off, off + pb)
                        psk = pss.tile([128, K], F32, tag="s", name="psk")
                        pairs = [(hT[:, kt, sl], s_qk[:, kt, :]) for kt in range(4)]
                        pairs.append((ones_bf[:, :pb], s_qkb[:]))
                        emit_group(psk[:pb, :], pairs)
                        nc.scalar.copy(qkbs[bt][:pb, t, :], psk[:pb, :])
                        psv = psg.tile([128, H], F32, tag="g", name="psv")
                        emit_group(psv[:pb, :],
                                   [(hT[:, kt, sl], s_qv[:, kt, :])
                                    for kt in range(4)])
                        if bt == 0:
                            nc.scalar.copy(qv_b0[:pb, t, :], psv[:pb, :])
                        else:
                            g4 = t % 4
                            nc.scalar.copy(
                                qv_p1[32 * g4:32 * (g4 + 1), t // 4, :],
                                psv[:pb, :])

            nc.vector.memset(hT[:], 0.0)
            nc.vector.memset(h_f[:], 0.0)

            # ---------- decoder ----------
            with tc.tile_pool(name="lg", bufs=1) as lg:
                s_gi = load(lg, w_gi, BF16)
                s_gh = load(lg, w_gh, BF16)
                s_out = load(lg, w_out, BF16)
                o19T = lg.tile([128, 3, BS], BF16)
                nc.vector.memset(o19T[32:64, 2, :], 0.0)
                nc.vector.memset(o19T[64:65, 2, :], 1.0)
                o19_0 = lg.tile([128, D], F32)
                o19_1 = lg.tile([128, D], F32)
                o19_sb = [o19_0, o19_1]
                logit_sb = lg.tile([128, VP], BF16)

                for t in range(MAX_LEN):
                    a_bf = st.tile([128, 2, K], BF16, name="a_bf")
                    aT = st.tile([128, BS], BF16, name="aT")
                    for bt in range(2):
                        pb, off = PBS[bt], BOFF[bt]
                        sl = slice(off, off + pb)
                        psa = pss.tile([128, K], F32, tag="s", name="psa")
                        pairs = [(hT[:, kt, sl], s_ak[:, kt, :]) for kt in range(4)]
                        pairs.append((ones_bf[:, :pb], s_akb[:]))
                        emit_group(psa[:pb, :], pairs)
                        nc.scalar.copy(a_bf[:pb, bt, :], psa[:pb, :])
                        tr(aT[:K, off:off + pb], a_bf[:pb, bt, :], pb, K)

                    qw_bf = st.tile([128, 2, L], BF16, name="qw_bf")
                    iwT = st.tile([128, 2, BS], BF16, name="iwT")
                    for bt in range(2):
                        pb, off = PBS[bt], BOFF[bt]
                        prod = wk.tile([128, L, K], BF16, tag="prod", bufs=3, name="prod")
                        peng = nc.vector if bt == 0 else nc.gpsimd
                        peng.tensor_mul(prod[:pb], qkbs[bt][:pb],
                                        bcast_mid(a_bf[:pb, bt, :], L))
                        qe = st.tile([128, L], F32, name="qe")
                        nc.vector.tensor_reduce(qe[:pb], prod[:pb], axis=AX.X,
                                                op=ALU.add)
                        nc.vector.tensor_add(qe[:pb], qe[:pb], s_qem[:pb, bt, :])
                        ew = st.tile([128, L], F32, name="ew")
                        ssum = st.tile([128, 1], F32, name="ssum")
                        nc.scalar.activation(ew[:pb], qe[:pb], AF.Exp,
                                             scale=1.0, accum_out=ssum[:pb])
                        rs = st.tile([128, 1], F32, name="rs")
                        nc.vector.reciprocal(rs[:pb], ssum[:pb])
                        nc.vector.tensor_scalar_mul(qw_bf[:pb, bt, :], ew[:pb],
                                                    rs[:pb])

                        psi = psg.tile([128, IL], F32, tag="g", name="psi")
                        nc.tensor.matmul(psi[:pb, :],
                                         aT[:K, off:off + pb],
                                         ikt[:K, :], start=True, stop=True)
                        iem = st.tile([128, IL], F32, tag="iem", name="iem")
                        nc.vector.tensor_add(iem[:pb], psi[:pb], s_iem[:pb, bt, :])
                        ewi = st.tile([128, IL], F32, tag="ewi", name="ewi")
                        ssi = st.tile([128, 1], F32, name="ssi")
                        nc.scalar.activation(ewi[:pb], iem[:pb], AF.Exp,
                                             scale=1.0, accum_out=ssi[:pb])
                        rsi = st.tile([128, 1], F32, name="rsi")
                        nc.vector.reciprocal(rsi[:pb], ssi[:pb])
                        iwb = st.tile([128, IL], BF16, name="iwb")
                        nc.vector.tensor_scalar_mul(iwb[:pb], ewi[:pb], rsi[:pb])
                        for c in range(2):
                            tr(iwT[:, c, off:off + pb],
                               iwb[:pb, c * 128:(c + 1) * 128], pb, 128)

                    for bt in range(2):
                        pb, off = PBS[bt], BOFF[bt]
                        psqc = psg.tile([128, H], F32, tag="g", name="psqc")
                        if bt == 0:
                            dg = wk.tile([128, L, 128], BF16, tag="diag",
                                         name="dg")
                            hl = L // 2
                            ibh = ident_bf[:pb, :pb]
                            ident_h = bass.AP(tensor=ibh.tensor, offset=ibh.offset,
                                              ap=[ibh.ap[0], [0, hl], ibh.ap[1]])
                            nc.gpsimd.tensor_mul(dg[:pb, :hl, :pb],
                                                 bcast_in(qw_bf[:pb, bt, :hl], pb),
                                                 ident_h)
                            nc.vector.tensor_mul(dg[:pb, hl:, :pb],
                                                 bcast_in(qw_bf[:pb, bt, hl:], pb),
                                                 ident_h)
                            for l in range(L):
                                nc.tensor.matmul(psqc[:pb, :], dg[:pb, l, :pb],
                                                 qv_b0[:pb, l, :],
                                                 start=(l == 0), stop=(l == L - 1))
                        else:
                            qw_pk = st.tile([128, 5], BF16, name="qw_pk")
                            for g4 in range(4):
                                nc.vector.tensor_copy(
                                    qw_pk[32 * g4:32 * (g4 + 1), :],
                                    qw_bf[0:32, 1, g4:L:4])
                            dg1 = wk.tile([128, 5, 32], BF16, tag="dg1",
                                          name="dg1")
                            sid_b = bass.AP(tensor=sid4.tensor,
                                            offset=sid4[:, :].offset,
                                            ap=[sid4[:, :].ap[0], [0, 5],
                                                sid4[:, :].ap[1]])
                            nc.vector.tensor_mul(dg1[:, :, :],
                                                 bcast_in(qw_pk[:, :], 32),
                                                 sid_b)
                            for c in range(5):
                                nc.tensor.matmul(psqc[:pb, :], dg1[:, c, :],
                                                 qv_p1[:, c, :],
                                                 start=(c == 0), stop=(c == 4))
                        qcb = st.tile([128, H], BF16, name="qcb")
                        nc.scalar.copy(qcb[:pb], psqc[:pb])
                        for kt in range(4):
                            tr_add(qcT[:, kt, off:off + pb],
                                   qcb[:pb, kt * 128:(kt + 1) * 128],
                                   s_qvb[:, kt:kt + 1], pb, 128)

                    for ht in range(4):
                        psic = pss.tile([128, BS], F32, tag="s", name="psic")
                        emit_group(psic[:, :],
                                   [(ivv[:, kt, ht * 128:(ht + 1) * 128],
                                     iwT[:, kt, :]) for kt in range(2)])
                        nc.vector.tensor_scalar_add(icT[:, ht, :], psic[:, :],
                                                    s_ivb[:, ht:ht + 1])

                    if t < L:
                        xt = fetch_x(s_aidx, t)
                        xsrc = ColView(xt, 0)
                    else:
                        xsrc = ColView(dec20, 0)
                    for bt in range(2):
                        gru_bt(bt, xsrc, [128, 128, 65], s_gi, s_gh, s_bhh,
                               [(qcT, k, 3 + k) for k in range(4)]
                               + [(icT, k, 7 + k) for k in range(4)],
                               tform=True)

                    for bt in range(2):
                        pb, off = PBS[bt], BOFF[bt]
                        sl = slice(off, off + pb)
                        pso = pss.tile([128, D], F32, tag="s", name="pso")
                        pairs = [(hT[:, k, sl], s_out[:, k, :]) for k in range(4)]
                        pairs += [(qcT[:, k, sl], s_out[:, 4 + k, :])
                                  for k in range(4)]
                        pairs += [(icT[:, k, sl], s_out[:, 8 + k, :])
                                  for k in range(4)]
                        pairs.append((ones_bf[:, :pb], s_outb[:]))
                        emit_group(pso[:pb, :], pairs)
                        osb = st.tile([128, D], F32, name="osb")
                        nc.scalar.copy(osb[:pb], pso[:pb])
                        nc.sync.dma_start(out_o[t, off:off + pb, :], osb[:pb])
                        if t == MAX_LEN - 2:
                            nc.vector.tensor_copy(o19_sb[bt][:pb], osb[:pb])

                    if t == MAX_LEN - 2:
                        for bt in range(2):
                            pb, off = PBS[bt], BOFF[bt]
                            ob = st.tile([128, D], BF16, name="ob")
                            nc.scalar.copy(ob[:pb], o19_sb[bt][:pb])
                            for kt in range(3):
                                w = 128 if kt < 2 else D - 256
                                tr(o19T[:w, kt, off:off + pb],
                                   ob[:pb, kt * 128:kt * 128 + w], pb, w)
                        for bt in range(2):
                            pb, off = PBS[bt], BOFF[bt]
                            for nci in range(18):
                                ncw = 512 if nci < 17 else V - 17 * 512
                                rhs = wk.tile([128, 3, 512], BF16, tag="lrhs", bufs=4,
                                              name="rhs")
                                for kt in range(3):
                                    nr = 128 if kt < 2 else 65
                                    nc.sync.dma_start(
                                        rhs[:nr, kt, :ncw],
                                        embt_bf[:nr, kt,
                                                nci * 512:nci * 512 + ncw])
                                psl = psg.tile([128, H], F32, tag="g", name="psl")
                                pairs = []
                                for kt in range(3):
                                    nr = 128 if kt < 2 else 65
                                    pairs.append((o19T[:nr, kt, off:off + pb],
                                                  rhs[:nr, kt, :ncw]))
                                emit_group(psl[:pb, :ncw], pairs)
                                nc.scalar.copy(
                                    logit_sb[:pb, nci * 512:nci * 512 + ncw],
                                    psl[:pb, :ncw])
                            if bt == 0:
                                nc.vector.memset(logit_sb[:, V:], -60000.0)
                            mx8 = st.tile([128, 8], BF16, name="mx8")
                            nc.vector.max(mx8[:pb], logit_sb[:pb])
                            ix8 = st.tile([128, 8], U32, name="ix8")
                            nc.vector.max_index(ix8[:pb], mx8[:pb], logit_sb[:pb])
                            scores = st.tile([128, 8], F32, name="scores")
                            for j in range(8):
                                g8 = wk.tile([128, D + 1], F32, tag="gath8",
                                             name="g8")
                                nc.gpsimd.indirect_dma_start(
                                    out=g8[:pb], out_offset=None, in_=emb_aug[:],
                                    in_offset=bass.IndirectOffsetOnAxis(
                                        ap=ix8[:pb, j:j + 1], axis=0))
                                pr = wk.tile([128, D], F32, tag="pr8", name="pr")
                                nc.vector.tensor_mul(pr[:pb], o19_sb[bt][:pb],
                                                     g8[:pb, :D])
                                sj = st.tile([128, 1], F32, name="sj")
                                nc.vector.tensor_reduce(sj[:pb], pr[:pb],
                                                        axis=AX.X, op=ALU.add)
                                nc.vector.tensor_add(scores[:pb, j:j + 1],
                                                     sj[:pb], g8[:pb, D:D + 1])
                            m1 = st.tile([128, 8], F32, name="m1")
                            nc.vector.max(m1[:pb], scores[:pb])
                            j1 = st.tile([128, 8], U32, name="j1")
                            nc.vector.max_index(j1[:pb], m1[:pb], scores[:pb])
                            j1f = st.tile([128, 1], F32, name="j1f")
                            nc.vector.tensor_copy(j1f[:pb], j1[:pb, 0:1])
                            oh = st.tile([128, 8], F32, name="oh")
                            nc.vector.tensor_scalar(out=oh[:pb], in0=iota8[:pb],
                                                    scalar1=j1f[:pb], scalar2=None,
                                                    op0=ALU.is_equal)
                            ix8f = st.tile([128, 8], F32, name="ix8f")
                            nc.vector.tensor_copy(ix8f[:pb], ix8[:pb])
                            nc.vector.tensor_mul(ix8f[:pb], oh[:pb], ix8f[:pb])
                            vsum = st.tile([128, 1], F32, name="vsum")
                            nc.vector.tensor_reduce(vsum[:pb], ix8f[:pb],
                                                    axis=AX.X, op=ALU.add)
                            vidx = st.tile([128, 1], U32, name="vidx")
                            nc.vector.tensor_copy(vidx[:pb], vsum[:pb])
                            gm = wk.tile([128, D], BF16, tag="gath", bufs=6,
                                         name="gm")
                            nc.gpsimd.indirect_dma_start(
                                out=gm[:pb], out_offset=None, in_=emb_bf[:],
                                in_offset=bass.IndirectOffsetOnAxis(
                                    ap=vidx[:pb, 0:1], axis=0))
                            for kt in range(3):
                                w = 128 if kt < 2 else D - 256
                                tr(dec20[:w, kt, off:off + pb],
                                   gm[:pb, kt * 128:kt * 128 + w], pb, w)

    nc.compile()
    return nc


_NC_CACHE = None


def _get_nc():
    global _NC_CACHE
    if _NC_CACHE is None:
        _NC_CACHE = build_nc()
    return _NC_CACHE


def _pad_tiles(a, ntiles):
    rows, cols = a.shape
    out = np.zeros((128 * ntiles, cols), a.dtype)
    out[:rows] = a
    return np.ascontiguousarray(out.reshape(ntiles, 128, cols).transpose(1, 0, 2))


def _prep_shared(inputs):
    bf = np.float16
    f32 = np.float32
    eW = np.asarray(inputs["embed_W"], f32)
    d = {}
    wih = np.asarray(inputs["dec_W_ih"], f32)
    bih = np.asarray(inputs["dec_b_ih"], f32)
    bhh = np.asarray(inputs["dec_b_hh"], f32)
    gi = np.zeros((128 * 11, 3 * H), f32)
    gi[0:D] = wih[:, 0:D].T
    gi[320] = bih + np.concatenate([bhh[:2 * H], np.zeros(H, f32)])
    gi[384:384 + H] = wih[:, D:D + H].T
    gi[896:896 + H] = wih[:, D + H:].T
    gi[:, 0:2 * H] *= 0.5
    gi[:, 2 * H:] *= 2.0
    d["w_gi"] = _pad_tiles(gi.astype(bf), 11)
    gh = np.asarray(inputs["dec_W_hh"], f32).T.copy()
    gh[:, 0:2 * H] *= 0.5
    d["w_gh"] = _pad_tiles(gh.astype(bf), 4)
    d["bhh_n"] = np.ascontiguousarray(bhh[2 * H:].astype(bf)[None, :])
    ewih = np.asarray(inputs["enc_W_ih"], f32)
    ebih = np.asarray(inputs["enc_b_ih"], f32)
    ebhh = np.asarray(inputs["enc_b_hh"], f32)
    egi = np.zeros((128 * 3, 3 * H), f32)
    egi[0:D] = ewih[:, :D].T
    egi[320] = ebih + np.concatenate([ebhh[:2 * H], np.zeros(H, f32)])
    d["w_egi"] = _pad_tiles(egi.astype(bf), 3)
    d["w_egh"] = _pad_tiles(np.asarray(inputs["enc_W_hh"], f32).T.astype(bf), 4)
    d["ebhh_n"] = np.ascontiguousarray(ebhh[2 * H:].astype(bf)[None, :])
    d["w_out"] = _pad_tiles(np.asarray(inputs["out_W"], f32).T.astype(bf), 12)
    d["outb"] = np.ascontiguousarray(
        np.asarray(inputs["out_b"], f32).astype(bf)[None, :])
    d["w_qk"] = _pad_tiles(np.asarray(inputs["qk_W"], f32).T.astype(bf), 4)
    d["qkb"] = np.ascontiguousarray(
        np.asarray(inputs["qk_b"], f32).astype(bf)[None, :])
    d["w_qv"] = _pad_tiles(np.asarray(inputs["qv_W"], f32).T.astype(bf), 4)
    d["qvb_c"] = np.ascontiguousarray(
        np.asarray(inputs["qv_b"], f32).reshape(4, 128).T)
    d["w_ak"] = _pad_tiles(np.asarray(inputs["ak_W"], f32).T.astype(bf), 4)
    d["akb"] = np.ascontiguousarray(
        np.asarray(inputs["ak_b"], f32).astype(bf)[None, :])
    d["w_ik"] = _pad_tiles(np.asarray(inputs["ik_W"], f32).T.astype(bf), 2)
    ikb = np.zeros((128, 1), f32)
    ikb[:K, 0] = np.asarray(inputs["ik_b"], f32)
    d["ikb_c"] = ikb
    d["w_iv"] = _pad_tiles(np.asarray(inputs["iv_W"], f32).T.astype(bf), 2)
    d["ivb_c"] = np.ascontiguousarray(
        np.asarray(inputs["iv_b"], f32).reshape(4, 128).T)
    d["emb_bf"] = eW.astype(bf)
    wd_b = np.asarray(inputs["wd_b"], f32)
    d["emb_aug"] = np.ascontiguousarray(np.concatenate([eW, wd_b[:, None]], 1))
    aug = np.zeros((128 * 3, VP), f32)
    aug[:D, :V] = eW.T
    aug[320, :V] = wd_b
    d["embt_bf"] = _pad_tiles(aug.astype(bf), 3)
    return d


def _idx_cols(seq_rows):
    out = np.zeros((128, 2 * L), np.uint32)
    for t in range(L):
        out[:, 2 * t] = seq_rows[0:128, t]
        out[:32, 2 * t + 1] = seq_rows[128:160, t]
    return out


def _build_maps(inputs, shared):
    f32 = np.float32
    bf = np.float16
    ques = np.asarray(inputs["ques_seqs"]).astype(np.uint32)
    ans = np.asarray(inputs["ans_seqs"]).astype(np.uint32)
    qlens = np.asarray(inputs["ques_lens"]).astype(np.int64)
    img = np.asarray(inputs["img_seqs"], f32)
    maps = []
    for s in range(NCORES):
        m = dict(shared)
        r0 = s * BS
        m["q_idx"] = _idx_cols(ques[r0:r0 + BS, :L])
        m["a_idx"] = _idx_cols(ans[r0:r0 + BS, :L])
        qm = np.full((128, 2, L), NEG, f32)
        lens = qlens[r0:r0 + BS]
        for bt, (pb, off) in enumerate(zip(PBS, BOFF)):
            for b in range(pb):
                qm[b, bt, :lens[off + b]] = 0.0
        m["qe_mask"] = qm
        im = np.full((128, 2, IL), NEG, f32)
        for bt, (pb, off) in enumerate(zip(PBS, BOFF)):
            for b in range(pb):
                gimg = (off + b) // ROUNDS
                im[b, bt, gimg * 16:(gimg + 1) * 16] = 0.0
        m["ie_mask"] = im
        imgs = img[s * 16:(s + 1) * 16].reshape(IL, 256)
        it = np.zeros((128 * 2, IL), f32)
        it[:256] = imgs.T
        m["img_t"] = np.ascontiguousarray(
            it.reshape(2, 128, IL).transpose(1, 0, 2)).astype(bf)
        maps.append(m)
    return maps


def kernel(**inputs):
    nc = _get_nc()
    shared = _prep_shared(inputs)
    in_maps = _build_maps(inputs, shared)
    from concourse.bass_utils import run_bass_kernel_spmd
    res = run_bass_kernel_spmd(nc, in_maps, core_ids=list(range(NCORES)))
    outs = []
    for s in range(NCORES):
        o = np.asarray(res.results[s]["out_o"])
        outs.append(np.ascontiguousarray(o.transpose(1, 0, 2)))
    return np.concatenate(outs, 0).astype(np.float32)



# revision 22
# speedup vs baseline: 1.0302x; 1.0302x over previous
"""Trainium2 Bass kernel for nn_BaselineAttnDecoder.

Data-parallel over 8 NeuronCores: each core handles 160 decode rows
(= 16 images x 10 rounds). All weights replicated. Per core:
  - question-encoder GRU (20 sequential steps, batch 160)
  - attention decoder GRU (21 sequential steps) with question + image
    attention, argmax re-embedding at step 19 (bf16 top-8 logits with
    exact f32 rescoring).

Layout: batch-stationary matmuls (lhsT = transposed activations
[feat, b], moving rhs = weight columns, N<=512) in bf16 with f32 PSUM
accumulation; gate math in f32; biases folded via augmented ones-rows,
rank-1 matmuls, and softmax-sum identities. Embedding rows gathered
just-in-time per step via indirect DMA and transposed on the PE.
"""
import numpy as np
import ml_dtypes

import concourse.bass as bass
import concourse.bacc as bacc
import concourse.mybir as mybir
import concourse.tile as tile
from concourse.masks import make_identity

F32 = mybir.dt.float32
BF16 = mybir.dt.float16  # 16-bit compute dtype (f16: 10-bit mantissa)
U32 = mybir.dt.uint32
AF = mybir.ActivationFunctionType
ALU = mybir.AluOpType
AX = mybir.AxisListType

D, H, V, K = 300, 512, 8835, 50
L, MAX_LEN, ROUNDS = 20, 21, 10
BS = 160
NCORES = 8
PBS = [128, 32]
BOFF = [0, 128]
IL = 256
VP = 18 * 512
NEG = -1.0e30


def bcast_mid(ap, reps):
    return bass.AP(tensor=ap.tensor, offset=ap.offset,
                   ap=[ap.ap[0], [0, reps], ap.ap[1]])


def bcast_in(ap, reps):
    return bass.AP(tensor=ap.tensor, offset=ap.offset,
                   ap=[ap.ap[0], ap.ap[1], [0, reps]])


class ColView:
    def __init__(self, base, col0):
        self.base, self.col0 = base, col0

    def __getitem__(self, key):
        rows, kt, sl = key
        return self.base[rows, kt, self.col0 + sl.start:self.col0 + sl.stop]


def build_nc():
    nc = bacc.Bacc()

    def din(name, shape, dt):
        return nc.dram_tensor(name, shape, dt, kind="ExternalInput")

    w_gi = din("w_gi", [128, 11, 3 * H], BF16)
    w_gh = din("w_gh", [128, 4, 3 * H], BF16)
    bhh_n = din("bhh_n", [1, H], BF16)
    w_egi = din("w_egi", [128, 3, 3 * H], BF16)
    w_egh = din("w_egh", [128, 4, 3 * H], BF16)
    ebhh_n = din("ebhh_n", [1, H], BF16)
    w_out = din("w_out", [128, 12, D], BF16)
    outb = din("outb", [1, D], BF16)
    w_qk = din("w_qk", [128, 4, K], BF16)
    qkb = din("qkb", [1, K], BF16)
    w_qv = din("w_qv", [128, 4, H], BF16)
    w_ak = din("w_ak", [128, 4, K], BF16)
    akb = din("akb", [1, K], BF16)
    w_ik = din("w_ik", [128, 2, K], BF16)
    ikb_c = din("ikb_c", [128, 1], F32)
    w_iv = din("w_iv", [128, 2, H], BF16)
    ivb_c = din("ivb_c", [128, 4], F32)
    qvb_c = din("qvb_c", [128, 4], F32)
    img_t = din("img_t", [128, 2, IL], BF16)
    emb_bf = din("emb_bf", [V, D], BF16)
    emb_aug = din("emb_aug", [V, D + 1], F32)
    embt_bf = din("embt_bf", [128, 3, VP], BF16)
    q_idx = din("q_idx", [128, 2 * L], U32)
    a_idx = din("a_idx", [128, 2 * L], U32)
    qe_mask = din("qe_mask", [128, 2, L], F32)
    ie_mask = din("ie_mask", [128, 2, IL], F32)

    out_o = nc.dram_tensor("out_o", [MAX_LEN, BS, D], F32, kind="ExternalOutput")

    with tile.TileContext(nc) as tc:
        with (
            tc.tile_pool(name="cw", bufs=1) as cw,
            tc.tile_pool(name="pers", bufs=1) as pers,
            tc.tile_pool(name="wk", bufs=2) as wk,
            tc.tile_pool(name="st", bufs=2) as st,
            tc.tile_pool(name="psg", bufs=5, space="PSUM") as psg,
            tc.tile_pool(name="pss", bufs=2, space="PSUM") as pss,
            tc.tile_pool(name="pst", bufs=1, space="PSUM") as pst,
        ):
            def load(pool, t, dt):
                s = pool.tile(list(t.shape), dt, name=t.name + "_sb")
                nc.sync.dma_start(s[:], t[:])
                return s

            s_qk = load(cw, w_qk, BF16)
            s_qv = load(cw, w_qv, BF16)
            s_ak = load(cw, w_ak, BF16)
            s_ik = load(cw, w_ik, BF16)
            s_iv = load(cw, w_iv, BF16)
            s_imgt = load(cw, img_t, BF16)
            s_bhh = load(cw, bhh_n, BF16)
            s_ebhh = load(cw, ebhh_n, BF16)
            s_outb = load(cw, outb, BF16)
            s_qkb = load(cw, qkb, BF16)
            s_akb = load(cw, akb, BF16)
            s_ikb = load(cw, ikb_c, F32)
            s_ivb = load(cw, ivb_c, F32)
            s_qvb = load(cw, qvb_c, F32)
            s_qidx = load(cw, q_idx, U32)
            s_aidx = load(cw, a_idx, U32)
            s_qem = load(cw, qe_mask, F32)
            s_iem = load(cw, ie_mask, F32)

            ident_bf = cw.tile([128, 128], BF16)
            make_identity(nc, ident_bf[:])
            ones_bf = cw.tile([1, 128], BF16)
            nc.vector.memset(ones_bf[:], 1.0)
            sid4 = cw.tile([128, 32], BF16)
            for g4 in range(4):
                nc.vector.tensor_copy(sid4[32 * g4:32 * (g4 + 1), :],
                                      ident_bf[0:32, 0:32])
            iota8 = cw.tile([128, 8], F32)
            nc.gpsimd.iota(iota8[:], pattern=[[1, 8]], base=0, channel_multiplier=0,
                           allow_small_or_imprecise_dtypes=True)

            hT = pers.tile([128, 4, BS], BF16)
            h_f = pers.tile([128, 2, H], F32)
            qk_b0 = pers.tile([128, L, K], BF16)
            qk_b1 = pers.tile([128, L, K], BF16)
            qv_b0 = pers.tile([128, L, H], BF16)
            qv_p1 = pers.tile([128, 5, H], BF16)
            qkbs = [qk_b0, qk_b1]
            ivv = pers.tile([128, 2, H], BF16)
            ikt = pers.tile([128, IL], BF16)
            qcT = pers.tile([128, 4, BS], BF16)
            icT = pers.tile([128, 4, BS], BF16)
            dec20 = pers.tile([128, 3, BS], BF16)

            nc.vector.memset(hT[:], 0.0)
            nc.vector.memset(h_f[:], 0.0)
            nc.vector.memset(dec20[32:64, 2, :], 0.0)
            nc.vector.memset(dec20[64:65, 2, :], 1.0)

            trctr = [0]

            def trslot():
                pt = pst.tile([128, 2, 128], BF16, tag="trp", name="pt")
                s = trctr[0] & 1
                trctr[0] += 1
                return pt[:, s, :]

            def tr(dst_sb_ap, src_sb_ap, pb, w, eng=None):
                pt = trslot()
                nc.tensor.transpose(pt[:w, :pb], src_sb_ap, ident_bf[:pb, :pb])
                (eng or nc.vector).tensor_copy(dst_sb_ap, pt[:w, :pb])

            def tr_add(dst_sb_ap, src_sb_ap, bias_ap, pb, w):
                pt = trslot()
                nc.tensor.transpose(pt[:w, :pb], src_sb_ap, ident_bf[:pb, :pb])
                nc.vector.tensor_scalar_add(dst_sb_ap, pt[:w, :pb], bias_ap)

            def fetch_x(idx_sb, t):
                xt = wk.tile([128, 3, BS], BF16, tag="xt", bufs=6, name="xt")
                nc.vector.memset(xt[32:64, 2, :], 0.0)
                nc.vector.memset(xt[64:65, 2, :], 1.0)
                for c, (pb, off) in enumerate(zip(PBS, BOFF)):
                    g = wk.tile([128, D], BF16, tag="gath", bufs=6, name="g")
                    nc.gpsimd.indirect_dma_start(
                        out=g[:pb], out_offset=None, in_=emb_bf[:],
                        in_offset=bass.IndirectOffsetOnAxis(
                            ap=idx_sb[:pb, 2 * t + c:2 * t + c + 1], axis=0))
                    for kt in range(3):
                        w = 128 if kt < 2 else D - 256
                        tr(xt[:w, kt, off:off + pb], g[:pb, kt * 128:kt * 128 + w],
                           pb, w)
                return xt

            def emit_group(ps_ap, pairs):
                n = len(pairs)
                for i, (lh, rh) in enumerate(pairs):
                    nc.tensor.matmul(ps_ap, lh, rh, start=(i == 0), stop=(i == n - 1))

            def gru_bt(bt, xsrc, xrows, w_rhs, gh_rhs, bhh_row, extra,
                       tform=False):
                pb, off = PBS[bt], BOFF[bt]
                sl = slice(off, off + pb)
                gps = []
                for ci in range(3):
                    cs = slice(ci * H, (ci + 1) * H)
                    ps = psg.tile([128, H], F32, tag="g", name="ps")
                    pairs = [(xsrc[slice(0, nr), kt, sl], w_rhs[:nr, kt, cs])
                             for kt, nr in enumerate(xrows)]
                    pairs += [(lt[:, lk, sl], w_rhs[:, rk, cs])
                              for (lt, lk, rk) in extra]
                    if ci < 2:
                        pairs += [(hT[:, kt, sl], gh_rhs[:, kt, cs])
                                  for kt in range(4)]
                    emit_group(ps[:pb, :], pairs)
                    gps.append(ps)
                ps_bn = psg.tile([128, H], F32, tag="g", name="ps_bn")
                pairs = [(hT[:, kt, sl], gh_rhs[:, kt, 2 * H:3 * H])
                         for kt in range(4)]
                pairs.append((ones_bf[:, :pb], bhh_row[:]))
                emit_group(ps_bn[:pb, :], pairs)

                ps_r, ps_z, ps_an = gps
                r = st.tile([128, H], F32, tag="r", bufs=1, name="r")
                z = st.tile([128, H], F32, tag="z", bufs=1, name="z")
                n = st.tile([128, H], F32, tag="n", bufs=1, name="n")
                t1 = st.tile([128, H], F32, tag="t1", bufs=1, name="t1")
                if tform:
                    # sigma(x) = (1 + tanh(x/2))/2 with 0.5/2.0 host-prescaled
                    # weights: keeps the whole decoder on the exp table set.
                    nc.scalar.activation(r[:pb], ps_r[:pb], AF.Tanh)
                    nc.scalar.activation(z[:pb], ps_z[:pb], AF.Tanh)
                    u = st.tile([128, H], F32, tag="u", bufs=1, name="u")
                    nc.gpsimd.tensor_scalar_add(u[:pb], r[:pb], 1.0)
                    nc.vector.tensor_mul(t1[:pb], u[:pb], ps_bn[:pb])
                    nc.vector.tensor_add(t1[:pb], t1[:pb], ps_an[:pb])
                    nc.scalar.activation(n[:pb], t1[:pb], AF.Tanh, scale=0.5)
                    zz = st.tile([128, H], F32, tag="zz", bufs=1, name="zz")
                    nc.vector.tensor_scalar(out=zz[:pb], in0=z[:pb],
                                            scalar1=0.5, scalar2=0.5,
                                            op0=ALU.mult, op1=ALU.add)
                    z = zz
                else:
                    nc.scalar.activation(r[:pb], ps_r[:pb], AF.Sigmoid)
                    nc.scalar.activation(z[:pb], ps_z[:pb], AF.Sigmoid)
                    nc.vector.tensor_mul(t1[:pb], r[:pb], ps_bn[:pb])
                    nc.vector.tensor_add(t1[:pb], t1[:pb], ps_an[:pb])
                    nc.scalar.activation(n[:pb], t1[:pb], AF.Tanh)
                d = st.tile([128, H], F32, tag="d", bufs=1, name="d")
                nc.gpsimd.tensor_sub(d[:pb], h_f[:pb, bt, :], n[:pb])
                nc.gpsimd.tensor_mul(d[:pb], d[:pb], z[:pb])
                nc.gpsimd.tensor_add(h_f[:pb, bt, :], d[:pb], n[:pb])
                hb = st.tile([128, H], BF16, tag="hb", bufs=1, name="hb")
                nc.scalar.copy(hb[:pb], h_f[:pb, bt, :])
                for kt in range(4):
                    tr(hT[:, kt, off:off + pb],
                       hb[:pb, kt * 128:(kt + 1) * 128], pb, 128)

            # ---------- image projections ----------
            for mt in range(2):
                psv = psg.tile([128, H], F32, tag="g", name="psv")
                emit_group(psv[:], [(s_imgt[:, kt, mt * 128:(mt + 1) * 128],
                                     s_iv[:, kt, :]) for kt in range(2)])
                nc.scalar.copy(ivv[:, mt, :], psv[:])
            psik = pss.tile([128, IL], F32, tag="s", name="psik")
            emit_group(psik[:K, :], [(s_ik[:, kt, :], s_imgt[:, kt, :])
                                     for kt in range(2)])
            nc.vector.tensor_scalar_add(ikt[:K, :], psik[:K, :], s_ikb[:K, :])

            # ---------- encoder ----------
            with tc.tile_pool(name="qp", bufs=1) as qp:
                s_egi = load(qp, w_egi, BF16)
                s_egh = load(qp, w_egh, BF16)
                for t in range(L):
                    xt = fetch_x(s_qidx, t)
                    for bt in range(2):
                        gru_bt(bt, ColView(xt, 0), [128, 128, 65],
                               s_egi, s_egh, s_ebhh, [])
                    for bt in range(2):
                        pb, off = PBS[bt], BOFF[bt]
                        sl = slice(off, off + pb)
                        psk = pss.tile([128, K], F32, tag="s", name="psk")
                        pairs = [(hT[:, kt, sl], s_qk[:, kt, :]) for kt in range(4)]
                        pairs.append((ones_bf[:, :pb], s_qkb[:]))
                        emit_group(psk[:pb, :], pairs)
                        nc.scalar.copy(qkbs[bt][:pb, t, :], psk[:pb, :])
                        psv = psg.tile([128, H], F32, tag="g", name="psv")
                        emit_group(psv[:pb, :],
                                   [(hT[:, kt, sl], s_qv[:, kt, :])
                                    for kt in range(4)])
                        if bt == 0:
                            nc.scalar.copy(qv_b0[:pb, t, :], psv[:pb, :])
                        else:
                            g4 = t % 4
                            nc.scalar.copy(
                                qv_p1[32 * g4:32 * (g4 + 1), t // 4, :],
                                psv[:pb, :])

            nc.vector.memset(hT[:], 0.0)
            nc.vector.memset(h_f[:], 0.0)

            # ---------- decoder ----------
            with tc.tile_pool(name="lg", bufs=1) as lg:
                s_gi = load(lg, w_gi, BF16)
                s_gh = load(lg, w_gh, BF16)
                s_out = load(lg, w_out, BF16)
                o19T = lg.tile([128, 3, BS], BF16)
                nc.vector.memset(o19T[32:64, 2, :], 0.0)
                nc.vector.memset(o19T[64:65, 2, :], 1.0)
                o19_0 = lg.tile([128, D], F32)
                o19_1 = lg.tile([128, D], F32)
                o19_sb = [o19_0, o19_1]
                logit_sb = lg.tile([128, VP], BF16)

                for t in range(MAX_LEN):
                    a_bf = st.tile([128, 2, K], BF16, name="a_bf")
                    aT = st.tile([128, BS], BF16, name="aT")
                    for bt in range(2):
                        pb, off = PBS[bt], BOFF[bt]
                        sl = slice(off, off + pb)
                        psa = pss.tile([128, K], F32, tag="s", name="psa")
                        pairs = [(hT[:, kt, sl], s_ak[:, kt, :]) for kt in range(4)]
                        pairs.append((ones_bf[:, :pb], s_akb[:]))
                        emit_group(psa[:pb, :], pairs)
                        nc.scalar.copy(a_bf[:pb, bt, :], psa[:pb, :])
                        tr(aT[:K, off:off + pb], a_bf[:pb, bt, :], pb, K)

                    qw_bf = st.tile([128, 2, L], BF16, name="qw_bf")
                    iwT = st.tile([128, 2, BS], BF16, name="iwT")
                    for bt in range(2):
                        pb, off = PBS[bt], BOFF[bt]
                        prod = wk.tile([128, L, K], BF16, tag="prod", bufs=3, name="prod")
                        peng = nc.vector if bt == 0 else nc.gpsimd
                        peng.tensor_mul(prod[:pb], qkbs[bt][:pb],
                                        bcast_mid(a_bf[:pb, bt, :], L))
                        qe = st.tile([128, L], F32, name="qe")
                        nc.vector.tensor_reduce(qe[:pb], prod[:pb], axis=AX.X,
                                                op=ALU.add)
                        nc.vector.tensor_add(qe[:pb], qe[:pb], s_qem[:pb, bt, :])
                        ew = st.tile([128, L], F32, name="ew")
                        ssum = st.tile([128, 1], F32, name="ssum")
                        nc.scalar.activation(ew[:pb], qe[:pb], AF.Exp,
                                             scale=1.0, accum_out=ssum[:pb])
                        rs = st.tile([128, 1], F32, name="rs")
                        nc.vector.reciprocal(rs[:pb], ssum[:pb])
                        nc.vector.tensor_scalar_mul(qw_bf[:pb, bt, :], ew[:pb],
                                                    rs[:pb])

                        psi = psg.tile([128, IL], F32, tag="g", name="psi")
                        nc.tensor.matmul(psi[:pb, :],
                                         aT[:K, off:off + pb],
                                         ikt[:K, :], start=True, stop=True)
                        iem = st.tile([128, IL], F32, tag="iem", name="iem")
                        nc.vector.tensor_add(iem[:pb], psi[:pb], s_iem[:pb, bt, :])
                        ewi = st.tile([128, IL], F32, tag="ewi", name="ewi")
                        ssi = st.tile([128, 1], F32, name="ssi")
                        nc.scalar.activation(ewi[:pb], iem[:pb], AF.Exp,
                                             scale=1.0, accum_out=ssi[:pb])
                        rsi = st.tile([128, 1], F32, name="rsi")
                        nc.vector.reciprocal(rsi[:pb], ssi[:pb])
                        iwb = st.tile([128, IL], BF16, name="iwb")
                        nc.vector.tensor_scalar_mul(iwb[:pb], ewi[:pb], rsi[:pb])
                        for c in range(2):
                            tr(iwT[:, c, off:off + pb],
                               iwb[:pb, c * 128:(c + 1) * 128], pb, 128)

                    for bt in range(2):
                        pb, off = PBS[bt], BOFF[bt]
                        psqc = psg.tile([128, H], F32, tag="g", name="psqc")
                        if bt == 0:
                            dg = wk.tile([128, L, 128], BF16, tag="diag",
                                         name="dg")
                            hl = L // 2
                            ibh = ident_bf[:pb, :pb]
                            ident_h = bass.AP(tensor=ibh.tensor, offset=ibh.offset,
                                              ap=[ibh.ap[0], [0, hl], ibh.ap[1]])
                            nc.gpsimd.tensor_mul(dg[:pb, :hl, :pb],
                                                 bcast_in(qw_bf[:pb, bt, :hl], pb),
                                                 ident_h)
                            nc.vector.tensor_mul(dg[:pb, hl:, :pb],
                                                 bcast_in(qw_bf[:pb, bt, hl:], pb),
                                                 ident_h)
                            for l in range(L):
                                nc.tensor.matmul(psqc[:pb, :], dg[:pb, l, :pb],
                                                 qv_b0[:pb, l, :],
                                                 start=(l == 0), stop=(l == L - 1))
                        else:
                            qw_pk = st.tile([128, 5], BF16, name="qw_pk")
                            for g4 in range(4):
                                nc.vector.tensor_copy(
                                    qw_pk[32 * g4:32 * (g4 + 1), :],
                                    qw_bf[0:32, 1, g4:L:4])
                            dg1 = wk.tile([128, 5, 32], BF16, tag="dg1",
                                          name="dg1")
                            sid_b = bass.AP(tensor=sid4.tensor,
                                            offset=sid4[:, :].offset,
                                            ap=[sid4[:, :].ap[0], [0, 5],
                                                sid4[:, :].ap[1]])
                            nc.vector.tensor_mul(dg1[:, :, :],
                                                 bcast_in(qw_pk[:, :], 32),
                                                 sid_b)
                            for c in range(5):
                                nc.tensor.matmul(psqc[:pb, :], dg1[:, c, :],
                                                 qv_p1[:, c, :],
                                                 start=(c == 0), stop=(c == 4))
                        qcb = st.tile([128, H], BF16, name="qcb")
                        nc.scalar.copy(qcb[:pb], psqc[:pb])
                        for kt in range(4):
                            tr_add(qcT[:, kt, off:off + pb],
                                   qcb[:pb, kt * 128:(kt + 1) * 128],
                                   s_qvb[:, kt:kt + 1], pb, 128)

                    for ht in range(4):
                        psic = pss.tile([128, BS], F32, tag="s", name="psic")
                        emit_group(psic[:, :],
                                   [(ivv[:, kt, ht * 128:(ht + 1) * 128],
                                     iwT[:, kt, :]) for kt in range(2)])
                        nc.vector.tensor_scalar_add(icT[:, ht, :], psic[:, :],
                                                    s_ivb[:, ht:ht + 1])

                    if t < L:
                        xt = fetch_x(s_aidx, t)
                        xsrc = ColView(xt, 0)
                    else:
                        xsrc = ColView(dec20, 0)
                    for bt in range(2):
                        gru_bt(bt, xsrc, [128, 128, 65], s_gi, s_gh, s_bhh,
                               [(qcT, k, 3 + k) for k in range(4)]
                               + [(icT, k, 7 + k) for k in range(4)],
                               tform=True)

                    for bt in range(2):
                        pb, off = PBS[bt], BOFF[bt]
                        sl = slice(off, off + pb)
                        pso = pss.tile([128, D], F32, tag="s", name="pso")
                        # ctx pairs first: they need no h', so the PE can run
                        # them while the gate-math chain produces hT
                        pairs = [(qcT[:, k, sl], s_out[:, 4 + k, :])
                                 for k in range(4)]
                        pairs += [(icT[:, k, sl], s_out[:, 8 + k, :])
                                  for k in range(4)]
                        pairs.append((ones_bf[:, :pb], s_outb[:]))
                        pairs += [(hT[:, k, sl], s_out[:, k, :]) for k in range(4)]
                        emit_group(pso[:pb, :], pairs)
                        osb = st.tile([128, D], F32, name="osb")
                        nc.scalar.copy(osb[:pb], pso[:pb])
                        nc.sync.dma_start(out_o[t, off:off + pb, :], osb[:pb])
                        if t == MAX_LEN - 2:
                            nc.vector.tensor_copy(o19_sb[bt][:pb], osb[:pb])

                    if t == MAX_LEN - 2:
                        for bt in range(2):
                            pb, off = PBS[bt], BOFF[bt]
                            ob = st.tile([128, D], BF16, name="ob")
                            nc.scalar.copy(ob[:pb], o19_sb[bt][:pb])
                            for kt in range(3):
                                w = 128 if kt < 2 else D - 256
                                tr(o19T[:w, kt, off:off + pb],
                                   ob[:pb, kt * 128:kt * 128 + w], pb, w)
                        for bt in range(2):
                            pb, off = PBS[bt], BOFF[bt]
                            for nci in range(18):
                                ncw = 512 if nci < 17 else V - 17 * 512
                                rhs = wk.tile([128, 3, 512], BF16, tag="lrhs", bufs=4,
                                              name="rhs")
                                for kt in range(3):
                                    nr = 128 if kt < 2 else 65
                                    nc.sync.dma_start(
                                        rhs[:nr, kt, :ncw],
                                        embt_bf[:nr, kt,
                                                nci * 512:nci * 512 + ncw])
                                psl = psg.tile([128, H], F32, tag="g", name="psl")
                                pairs = []
                                for kt in range(3):
                                    nr = 128 if kt < 2 else 65
                                    pairs.append((o19T[:nr, kt, off:off + pb],
                                                  rhs[:nr, kt, :ncw]))
                                emit_group(psl[:pb, :ncw], pairs)
                                nc.scalar.copy(
                                    logit_sb[:pb, nci * 512:nci * 512 + ncw],
                                    psl[:pb, :ncw])
                            if bt == 0:
                                nc.vector.memset(logit_sb[:, V:], -60000.0)
                            mx8 = st.tile([128, 8], BF16, name="mx8")
                            nc.vector.max(mx8[:pb], logit_sb[:pb])
                            ix8 = st.tile([128, 8], U32, name="ix8")
                            nc.vector.max_index(ix8[:pb], mx8[:pb], logit_sb[:pb])
                            scores = st.tile([128, 8], F32, name="scores")
                            for j in range(8):
                                g8 = wk.tile([128, D + 1], F32, tag="gath8",
                                             name="g8")
                                nc.gpsimd.indirect_dma_start(
                                    out=g8[:pb], out_offset=None, in_=emb_aug[:],
                                    in_offset=bass.IndirectOffsetOnAxis(
                                        ap=ix8[:pb, j:j + 1], axis=0))
                                pr = wk.tile([128, D], F32, tag="pr8", name="pr")
                                nc.vector.tensor_mul(pr[:pb], o19_sb[bt][:pb],
                                                     g8[:pb, :D])
                                sj = st.tile([128, 1], F32, name="sj")
                                nc.vector.tensor_reduce(sj[:pb], pr[:pb],
                                                        axis=AX.X, op=ALU.add)
                                nc.vector.tensor_add(scores[:pb, j:j + 1],
                                                     sj[:pb], g8[:pb, D:D + 1])
                            m1 = st.tile([128, 8], F32, name="m1")
                            nc.vector.max(m1[:pb], scores[:pb])
                            j1 = st.tile([128, 8], U32, name="j1")
                            nc.vector.max_index(j1[:pb], m1[:pb], scores[:pb])
                            j1f = st.tile([128, 1], F32, name="j1f")
                            nc.vector.tensor_copy(j1f[:pb], j1[:pb, 0:1])
                            oh = st.tile([128, 8], F32, name="oh")
                            nc.vector.tensor_scalar(out=oh[:pb], in0=iota8[:pb],
                                                    scalar1=j1f[:pb], scalar2=None,
                                                    op0=ALU.is_equal)
                            ix8f = st.tile([128, 8], F32, name="ix8f")
                            nc.vector.tensor_copy(ix8f[:pb], ix8[:pb])
                            nc.vector.tensor_mul(ix8f[:pb], oh[:pb], ix8f[:pb])
                            vsum = st.tile([128, 1], F32, name="vsum")
                            nc.vector.tensor_reduce(vsum[:pb], ix8f[:pb],
                                                    axis=AX.X, op=ALU.add)
                            vidx = st.tile([128, 1], U32, name="vidx")
                            nc.vector.tensor_copy(vidx[:pb], vsum[:pb])
                            gm = wk.tile([128, D], BF16, tag="gath", bufs=6,
                                         name="gm")
                            nc.gpsimd.indirect_dma_start(
                                out=gm[:pb], out_offset=None, in_=emb_bf[:],
                                in_offset=bass.IndirectOffsetOnAxis(
                                    ap=vidx[:pb, 0:1], axis=0))
                            for kt in range(3):
                                w = 128 if kt < 2 else D - 256
                                tr(dec20[:w, kt, off:off + pb],
                                   gm[:pb, kt * 128:kt * 128 + w], pb, w)

    nc.compile()
    return nc


_NC_CACHE = None


def _get_nc():
    global _NC_CACHE
    if _NC_CACHE is None:
        _NC_CACHE = build_nc()
    return _NC_CACHE


def _pad_tiles(a, ntiles):
    rows, cols = a.shape
    out = np.zeros((128 * ntiles, cols), a.dtype)
    out[:rows] = a
    return np.ascontiguousarray(out.reshape(ntiles, 128, cols).transpose(1, 0, 2))


def _prep_shared(inputs):
    bf = np.float16
    f32 = np.float32
    eW = np.asarray(inputs["embed_W"], f32)
    d = {}
    wih = np.asarray(inputs["dec_W_ih"], f32)
    bih = np.asarray(inputs["dec_b_ih"], f32)
    bhh = np.asarray(inputs["dec_b_hh"], f32)
    gi = np.zeros((128 * 11, 3 * H), f32)
    gi[0:D] = wih[:, 0:D].T
    gi[320] = bih + np.concatenate([bhh[:2 * H], np.zeros(H, f32)])
    gi[384:384 + H] = wih[:, D:D + H].T
    gi[896:896 + H] = wih[:, D + H:].T
    gi[:, 0:2 * H] *= 0.5
    gi[:, 2 * H:] *= 2.0
    d["w_gi"] = _pad_tiles(gi.astype(bf), 11)
    gh = np.asarray(inputs["dec_W_hh"], f32).T.copy()
    gh[:, 0:2 * H] *= 0.5
    d["w_gh"] = _pad_tiles(gh.astype(bf), 4)
    d["bhh_n"] = np.ascontiguousarray(bhh[2 * H:].astype(bf)[None, :])
    ewih = np.asarray(inputs["enc_W_ih"], f32)
    ebih = np.asarray(inputs["enc_b_ih"], f32)
    ebhh = np.asarray(inputs["enc_b_hh"], f32)
    egi = np.zeros((128 * 3, 3 * H), f32)
    egi[0:D] = ewih[:, :D].T
    egi[320] = ebih + np.concatenate([ebhh[:2 * H], np.zeros(H, f32)])
    d["w_egi"] = _pad_tiles(egi.astype(bf), 3)
    d["w_egh"] = _pad_tiles(np.asarray(inputs["enc_W_hh"], f32).T.astype(bf), 4)
    d["ebhh_n"] = np.ascontiguousarray(ebhh[2 * H:].astype(bf)[None, :])
    d["w_out"] = _pad_tiles(np.asarray(inputs["out_W"], f32).T.astype(bf), 12)
    d["outb"] = np.ascontiguousarray(
        np.asarray(inputs["out_b"], f32).astype(bf)[None, :])
    d["w_qk"] = _pad_tiles(np.asarray(inputs["qk_W"], f32).T.astype(bf), 4)
    d["qkb"] = np.ascontiguousarray(
        np.asarray(inputs["qk_b"], f32).astype(bf)[None, :])
    d["w_qv"] = _pad_tiles(np.asarray(inputs["qv_W"], f32).T.astype(bf), 4)
    d["qvb_c"] = np.ascontiguousarray(
        np.asarray(inputs["qv_b"], f32).reshape(4, 128).T)
    d["w_ak"] = _pad_tiles(np.asarray(inputs["ak_W"], f32).T.astype(bf), 4)
    d["akb"] = np.ascontiguousarray(
        np.asarray(inputs["ak_b"], f32).astype(bf)[None, :])
    d["w_ik"] = _pad_tiles(np.asarray(inputs["ik_W"], f32).T.astype(bf), 2)
    ikb = np.zeros((128, 1), f32)
    ikb[:K, 0] = np.asarray(inputs["ik_b"], f32)
    d["ikb_c"] = ikb
    d["w_iv"] = _pad_tiles(np.asarray(inputs["iv_W"], f32).T.astype(bf), 2)
    d["ivb_c"] = np.ascontiguousarray(
        np.asarray(inputs["iv_b"], f32).reshape(4, 128).T)
    d["emb_bf"] = eW.astype(bf)
    wd_b = np.asarray(inputs["wd_b"], f32)
    d["emb_aug"] = np.ascontiguousarray(np.concatenate([eW, wd_b[:, None]], 1))
    aug = np.zeros((128 * 3, VP), f32)
    aug[:D, :V] = eW.T
    aug[320, :V] = wd_b
    d["embt_bf"] = _pad_tiles(aug.astype(bf), 3)
    return d


def _idx_cols(seq_rows):
    out = np.zeros((128, 2 * L), np.uint32)
    for t in range(L):
        out[:, 2 * t] = seq_rows[0:128, t]
        out[:32, 2 * t + 1] = seq_rows[128:160, t]
    return out


def _build_maps(inputs, shared):
    f32 = np.float32
    bf = np.float16
    ques = np.asarray(inputs["ques_seqs"]).astype(np.uint32)
    ans = np.asarray(inputs["ans_seqs"]).astype(np.uint32)
    qlens = np.asarray(inputs["ques_lens"]).astype(np.int64)
    img = np.asarray(inputs["img_seqs"], f32)
    maps = []
    for s in range(NCORES):
        m = dict(shared)
        r0 = s * BS
        m["q_idx"] = _idx_cols(ques[r0:r0 + BS, :L])
        m["a_idx"] = _idx_cols(ans[r0:r0 + BS, :L])
        qm = np.full((128, 2, L), NEG, f32)
        lens = qlens[r0:r0 + BS]
        for bt, (pb, off) in enumerate(zip(PBS, BOFF)):
            for b in range(pb):
                qm[b, bt, :lens[off + b]] = 0.0
        m["qe_mask"] = qm
        im = np.full((128, 2, IL), NEG, f32)
        for bt, (pb, off) in enumerate(zip(PBS, BOFF)):
            for b in range(pb):
                gimg = (off + b) // ROUNDS
                im[b, bt, gimg * 16:(gimg + 1) * 16] = 0.0
        m["ie_mask"] = im
        imgs = img[s * 16:(s + 1) * 16].reshape(IL, 256)
        it = np.zeros((128 * 2, IL), f32)
        it[:256] = imgs.T
        m["img_t"] = np.ascontiguousarray(
            it.reshape(2, 128, IL).transpose(1, 0, 2)).astype(bf)
        maps.append(m)
    return maps


def kernel(**inputs):
    nc = _get_nc()
    shared = _prep_shared(inputs)
    in_maps = _build_maps(inputs, shared)
    from concourse.bass_utils import run_bass_kernel_spmd
    res = run_bass_kernel_spmd(nc, in_maps, core_ids=list(range(NCORES)))
    outs = []
    for s in range(NCORES):
        o = np.asarray(res.results[s]["out_o"])
        outs.append(np.ascontiguousarray(o.transpose(1, 0, 2)))
    return np.concatenate(outs, 0).astype(np.float32)



# revision 23
# speedup vs baseline: 1.0345x; 1.0042x over previous
"""Trainium2 Bass kernel for nn_BaselineAttnDecoder.

Data-parallel over 8 NeuronCores: each core handles 160 decode rows
(= 16 images x 10 rounds). All weights replicated. Per core:
  - question-encoder GRU (20 sequential steps, batch 160)
  - attention decoder GRU (21 sequential steps) with question + image
    attention, argmax re-embedding at step 19 (bf16 top-8 logits with
    exact f32 rescoring).

Layout: batch-stationary matmuls (lhsT = transposed activations
[feat, b], moving rhs = weight columns, N<=512) in bf16 with f32 PSUM
accumulation; gate math in f32; biases folded via augmented ones-rows,
rank-1 matmuls, and softmax-sum identities. Embedding rows gathered
just-in-time per step via indirect DMA and transposed on the PE.
"""
import numpy as np
import ml_dtypes

import concourse.bass as bass
import concourse.bacc as bacc
import concourse.mybir as mybir
import concourse.tile as tile
from concourse.masks import make_identity

F32 = mybir.dt.float32
BF16 = mybir.dt.float16  # 16-bit compute dtype (f16: 10-bit mantissa)
U32 = mybir.dt.uint32
AF = mybir.ActivationFunctionType
ALU = mybir.AluOpType
AX = mybir.AxisListType

D, H, V, K = 300, 512, 8835, 50
L, MAX_LEN, ROUNDS = 20, 21, 10
BS = 160
NCORES = 8
PBS = [128, 32]
BOFF = [0, 128]
IL = 256
VP = 18 * 512
NEG = -1.0e30


def bcast_mid(ap, reps):
    return bass.AP(tensor=ap.tensor, offset=ap.offset,
                   ap=[ap.ap[0], [0, reps], ap.ap[1]])


def bcast_in(ap, reps):
    return bass.AP(tensor=ap.tensor, offset=ap.offset,
                   ap=[ap.ap[0], ap.ap[1], [0, reps]])


class ColView:
    def __init__(self, base, col0):
        self.base, self.col0 = base, col0

    def __getitem__(self, key):
        rows, kt, sl = key
        return self.base[rows, kt, self.col0 + sl.start:self.col0 + sl.stop]


def build_nc():
    nc = bacc.Bacc()

    def din(name, shape, dt):
        return nc.dram_tensor(name, shape, dt, kind="ExternalInput")

    w_gi = din("w_gi", [128, 11, 3 * H], BF16)
    w_gh = din("w_gh", [128, 4, 3 * H], BF16)
    bhh_n = din("bhh_n", [1, H], BF16)
    w_egi = din("w_egi", [128, 3, 3 * H], BF16)
    w_egh = din("w_egh", [128, 4, 3 * H], BF16)
    ebhh_n = din("ebhh_n", [1, H], BF16)
    w_out = din("w_out", [128, 12, D], BF16)
    outb = din("outb", [1, D], BF16)
    w_qk = din("w_qk", [128, 4, K], BF16)
    qkb = din("qkb", [1, K], BF16)
    w_qv = din("w_qv", [128, 4, H], BF16)
    w_ak = din("w_ak", [128, 4, K], BF16)
    akb = din("akb", [1, K], BF16)
    w_ik = din("w_ik", [128, 2, K], BF16)
    ikb_c = din("ikb_c", [128, 1], F32)
    w_iv = din("w_iv", [128, 2, H], BF16)
    ivb_c = din("ivb_c", [128, 4], F32)
    qvb_c = din("qvb_c", [128, 4], F32)
    img_t = din("img_t", [128, 2, IL], BF16)
    emb_bf = din("emb_bf", [V, D], BF16)
    emb_aug = din("emb_aug", [V, D + 1], F32)
    embt_bf = din("embt_bf", [128, 3, VP], BF16)
    q_idx = din("q_idx", [128, 2 * L], U32)
    a_idx = din("a_idx", [128, 2 * L], U32)
    qe_mask = din("qe_mask", [128, 2, L], F32)
    ie_mask = din("ie_mask", [128, 2, IL], F32)

    out_o = nc.dram_tensor("out_o", [MAX_LEN, BS, D], F32, kind="ExternalOutput")

    with tile.TileContext(nc) as tc:
        with (
            tc.tile_pool(name="cw", bufs=1) as cw,
            tc.tile_pool(name="pers", bufs=1) as pers,
            tc.tile_pool(name="wk", bufs=2) as wk,
            tc.tile_pool(name="st", bufs=2) as st,
            tc.tile_pool(name="psg", bufs=5, space="PSUM") as psg,
            tc.tile_pool(name="pss", bufs=2, space="PSUM") as pss,
            tc.tile_pool(name="pst", bufs=1, space="PSUM") as pst,
        ):
            def load(pool, t, dt):
                s = pool.tile(list(t.shape), dt, name=t.name + "_sb")
                nc.sync.dma_start(s[:], t[:])
                return s

            s_qk = load(cw, w_qk, BF16)
            s_qv = load(cw, w_qv, BF16)
            s_ak = load(cw, w_ak, BF16)
            s_ik = load(cw, w_ik, BF16)
            s_iv = load(cw, w_iv, BF16)
            s_imgt = load(cw, img_t, BF16)
            s_bhh = load(cw, bhh_n, BF16)
            s_ebhh = load(cw, ebhh_n, BF16)
            s_outb = load(cw, outb, BF16)
            s_qkb = load(cw, qkb, BF16)
            s_akb = load(cw, akb, BF16)
            s_ikb = load(cw, ikb_c, F32)
            s_ivb = load(cw, ivb_c, F32)
            s_qvb = load(cw, qvb_c, F32)
            s_qidx = load(cw, q_idx, U32)
            s_aidx = load(cw, a_idx, U32)
            s_qem = load(cw, qe_mask, F32)
            s_iem = load(cw, ie_mask, F32)

            ident_bf = cw.tile([128, 128], BF16)
            make_identity(nc, ident_bf[:])
            ones_bf = cw.tile([1, 128], BF16)
            nc.vector.memset(ones_bf[:], 1.0)
            sid4 = cw.tile([128, 32], BF16)
            for g4 in range(4):
                nc.vector.tensor_copy(sid4[32 * g4:32 * (g4 + 1), :],
                                      ident_bf[0:32, 0:32])
            iota8 = cw.tile([128, 8], F32)
            nc.gpsimd.iota(iota8[:], pattern=[[1, 8]], base=0, channel_multiplier=0,
                           allow_small_or_imprecise_dtypes=True)

            hT = pers.tile([128, 4, BS], BF16)
            h_f = pers.tile([128, 2, H], F32)
            qk_b0 = pers.tile([128, L, K], BF16)
            qk_b1 = pers.tile([128, L, K], BF16)
            qv_b0 = pers.tile([128, L, H], BF16)
            qv_p1 = pers.tile([128, 5, H], BF16)
            qkbs = [qk_b0, qk_b1]
            ivv = pers.tile([128, 2, H], BF16)
            ikt = pers.tile([128, IL], BF16)
            qcT = pers.tile([128, 4, BS], BF16)
            icT = pers.tile([128, 4, BS], BF16)
            dec20 = pers.tile([128, 3, BS], BF16)

            nc.vector.memset(hT[:], 0.0)
            nc.vector.memset(h_f[:], 0.0)
            nc.vector.memset(dec20[32:64, 2, :], 0.0)
            nc.vector.memset(dec20[64:65, 2, :], 1.0)

            trctr = [0]

            def trslot():
                pt = pst.tile([128, 2, 128], BF16, tag="trp", name="pt")
                s = trctr[0] & 1
                trctr[0] += 1
                return pt[:, s, :]

            def tr(dst_sb_ap, src_sb_ap, pb, w, eng=None):
                pt = trslot()
                nc.tensor.transpose(pt[:w, :pb], src_sb_ap, ident_bf[:pb, :pb])
                (eng or nc.vector).tensor_copy(dst_sb_ap, pt[:w, :pb])

            def tr_add(dst_sb_ap, src_sb_ap, bias_ap, pb, w):
                pt = trslot()
                nc.tensor.transpose(pt[:w, :pb], src_sb_ap, ident_bf[:pb, :pb])
                nc.vector.tensor_scalar_add(dst_sb_ap, pt[:w, :pb], bias_ap)

            def fetch_x(idx_sb, t):
                xt = wk.tile([128, 3, BS], BF16, tag="xt", bufs=6, name="xt")
                nc.vector.memset(xt[32:64, 2, :], 0.0)
                nc.vector.memset(xt[64:65, 2, :], 1.0)
                for c, (pb, off) in enumerate(zip(PBS, BOFF)):
                    g = wk.tile([128, D], BF16, tag="gath", bufs=6, name="g")
                    nc.gpsimd.indirect_dma_start(
                        out=g[:pb], out_offset=None, in_=emb_bf[:],
                        in_offset=bass.IndirectOffsetOnAxis(
                            ap=idx_sb[:pb, 2 * t + c:2 * t + c + 1], axis=0))
                    for kt in range(3):
                        w = 128 if kt < 2 else D - 256
                        tr(xt[:w, kt, off:off + pb], g[:pb, kt * 128:kt * 128 + w],
                           pb, w)
                return xt

            def emit_group(ps_ap, pairs):
                n = len(pairs)
                for i, (lh, rh) in enumerate(pairs):
                    nc.tensor.matmul(ps_ap, lh, rh, start=(i == 0), stop=(i == n - 1))

            def gru_bt(bt, xsrc, xrows, w_rhs, gh_rhs, bhh_row, extra,
                       tform=False):
                pb, off = PBS[bt], BOFF[bt]
                sl = slice(off, off + pb)
                gps = []
                for ci in range(3):
                    cs = slice(ci * H, (ci + 1) * H)
                    ps = psg.tile([128, H], F32, tag="g", name="ps")
                    pairs = [(xsrc[slice(0, nr), kt, sl], w_rhs[:nr, kt, cs])
                             for kt, nr in enumerate(xrows)]
                    pairs += [(lt[:, lk, sl], w_rhs[:, rk, cs])
                              for (lt, lk, rk) in extra]
                    if ci < 2:
                        pairs += [(hT[:, kt, sl], gh_rhs[:, kt, cs])
                                  for kt in range(4)]
                    emit_group(ps[:pb, :], pairs)
                    gps.append(ps)
                ps_bn = psg.tile([128, H], F32, tag="g", name="ps_bn")
                pairs = [(hT[:, kt, sl], gh_rhs[:, kt, 2 * H:3 * H])
                         for kt in range(4)]
                pairs.append((ones_bf[:, :pb], bhh_row[:]))
                emit_group(ps_bn[:pb, :], pairs)

                ps_r, ps_z, ps_an = gps
                r = st.tile([128, H], F32, tag="r", bufs=1, name="r")
                z = st.tile([128, H], F32, tag="z", bufs=1, name="z")
                n = st.tile([128, H], F32, tag="n", bufs=1, name="n")
                t1 = st.tile([128, H], F32, tag="t1", bufs=1, name="t1")
                if tform:
                    # sigma(x) = (1 + tanh(x/2))/2 with 0.5/2.0 host-prescaled
                    # weights: keeps the whole decoder on the exp table set.
                    nc.scalar.activation(r[:pb], ps_r[:pb], AF.Tanh)
                    nc.scalar.activation(z[:pb], ps_z[:pb], AF.Tanh)
                    u = st.tile([128, H], F32, tag="u", bufs=1, name="u")
                    nc.gpsimd.tensor_scalar_add(u[:pb], r[:pb], 1.0)
                    nc.vector.tensor_mul(t1[:pb], u[:pb], ps_bn[:pb])
                    nc.vector.tensor_add(t1[:pb], t1[:pb], ps_an[:pb])
                    nc.scalar.activation(n[:pb], t1[:pb], AF.Tanh, scale=0.5)
                    zz = st.tile([128, H], F32, tag="zz", bufs=1, name="zz")
                    nc.vector.tensor_scalar(out=zz[:pb], in0=z[:pb],
                                            scalar1=0.5, scalar2=0.5,
                                            op0=ALU.mult, op1=ALU.add)
                    z = zz
                else:
                    nc.scalar.activation(r[:pb], ps_r[:pb], AF.Sigmoid)
                    nc.scalar.activation(z[:pb], ps_z[:pb], AF.Sigmoid)
                    nc.vector.tensor_mul(t1[:pb], r[:pb], ps_bn[:pb])
                    nc.vector.tensor_add(t1[:pb], t1[:pb], ps_an[:pb])
                    nc.scalar.activation(n[:pb], t1[:pb], AF.Tanh)
                d = st.tile([128, H], F32, tag="d", bufs=1, name="d")
                nc.gpsimd.tensor_sub(d[:pb], h_f[:pb, bt, :], n[:pb])
                nc.gpsimd.tensor_mul(d[:pb], d[:pb], z[:pb])
                nc.gpsimd.tensor_add(h_f[:pb, bt, :], d[:pb], n[:pb])
                hb = st.tile([128, H], BF16, tag="hb", bufs=1, name="hb")
                nc.vector.tensor_copy(hb[:pb], h_f[:pb, bt, :])
                for kt in range(4):
                    tr(hT[:, kt, off:off + pb],
                       hb[:pb, kt * 128:(kt + 1) * 128], pb, 128)

            # ---------- image projections ----------
            for mt in range(2):
                psv = psg.tile([128, H], F32, tag="g", name="psv")
                emit_group(psv[:], [(s_imgt[:, kt, mt * 128:(mt + 1) * 128],
                                     s_iv[:, kt, :]) for kt in range(2)])
                nc.scalar.copy(ivv[:, mt, :], psv[:])
            psik = pss.tile([128, IL], F32, tag="s", name="psik")
            emit_group(psik[:K, :], [(s_ik[:, kt, :], s_imgt[:, kt, :])
                                     for kt in range(2)])
            nc.vector.tensor_scalar_add(ikt[:K, :], psik[:K, :], s_ikb[:K, :])

            # ---------- encoder ----------
            with tc.tile_pool(name="qp", bufs=1) as qp:
                s_egi = load(qp, w_egi, BF16)
                s_egh = load(qp, w_egh, BF16)
                for t in range(L):
                    xt = fetch_x(s_qidx, t)
                    for bt in range(2):
                        gru_bt(bt, ColView(xt, 0), [128, 128, 65],
                               s_egi, s_egh, s_ebhh, [])
                    for bt in range(2):
                        pb, off = PBS[bt], BOFF[bt]
                        sl = slice(off, off + pb)
                        psk = pss.tile([128, K], F32, tag="s", name="psk")
                        pairs = [(hT[:, kt, sl], s_qk[:, kt, :]) for kt in range(4)]
                        pairs.append((ones_bf[:, :pb], s_qkb[:]))
                        emit_group(psk[:pb, :], pairs)
                        nc.scalar.copy(qkbs[bt][:pb, t, :], psk[:pb, :])
                        psv = psg.tile([128, H], F32, tag="g", name="psv")
                        emit_group(psv[:pb, :],
                                   [(hT[:, kt, sl], s_qv[:, kt, :])
                                    for kt in range(4)])
                        if bt == 0:
                            nc.scalar.copy(qv_b0[:pb, t, :], psv[:pb, :])
                        else:
                            g4 = t % 4
                            nc.scalar.copy(
                                qv_p1[32 * g4:32 * (g4 + 1), t // 4, :],
                                psv[:pb, :])

            nc.vector.memset(hT[:], 0.0)
            nc.vector.memset(h_f[:], 0.0)

            # ---------- decoder ----------
            with tc.tile_pool(name="lg", bufs=1) as lg:
                s_gi = load(lg, w_gi, BF16)
                s_gh = load(lg, w_gh, BF16)
                s_out = load(lg, w_out, BF16)
                o19T = lg.tile([128, 3, BS], BF16)
                nc.vector.memset(o19T[32:64, 2, :], 0.0)
                nc.vector.memset(o19T[64:65, 2, :], 1.0)
                o19_0 = lg.tile([128, D], F32)
                o19_1 = lg.tile([128, D], F32)
                o19_sb = [o19_0, o19_1]
                logit_sb = lg.tile([128, VP], BF16)

                for t in range(MAX_LEN):
                    a_bf = st.tile([128, 2, K], BF16, name="a_bf")
                    aT = st.tile([128, BS], BF16, name="aT")
                    for bt in range(2):
                        pb, off = PBS[bt], BOFF[bt]
                        sl = slice(off, off + pb)
                        psa = pss.tile([128, K], F32, tag="s", name="psa")
                        pairs = [(hT[:, kt, sl], s_ak[:, kt, :]) for kt in range(4)]
                        pairs.append((ones_bf[:, :pb], s_akb[:]))
                        emit_group(psa[:pb, :], pairs)
                        nc.scalar.copy(a_bf[:pb, bt, :], psa[:pb, :])
                        tr(aT[:K, off:off + pb], a_bf[:pb, bt, :], pb, K)

                    qw_bf = st.tile([128, 2, L], BF16, name="qw_bf")
                    iwT = st.tile([128, 2, BS], BF16, name="iwT")
                    for bt in range(2):
                        pb, off = PBS[bt], BOFF[bt]
                        prod = wk.tile([128, L, K], BF16, tag="prod", bufs=3, name="prod")
                        peng = nc.vector if bt == 0 else nc.gpsimd
                        peng.tensor_mul(prod[:pb], qkbs[bt][:pb],
                                        bcast_mid(a_bf[:pb, bt, :], L))
                        qe = st.tile([128, L], F32, name="qe")
                        nc.vector.tensor_reduce(qe[:pb], prod[:pb], axis=AX.X,
                                                op=ALU.add)
                        nc.vector.tensor_add(qe[:pb], qe[:pb], s_qem[:pb, bt, :])
                        ew = st.tile([128, L], F32, name="ew")
                        ssum = st.tile([128, 1], F32, name="ssum")
                        nc.scalar.activation(ew[:pb], qe[:pb], AF.Exp,
                                             scale=1.0, accum_out=ssum[:pb])
                        rs = st.tile([128, 1], F32, name="rs")
                        nc.vector.reciprocal(rs[:pb], ssum[:pb])
                        nc.vector.tensor_scalar_mul(qw_bf[:pb, bt, :], ew[:pb],
                                                    rs[:pb])

                        psi = psg.tile([128, IL], F32, tag="g", name="psi")
                        nc.tensor.matmul(psi[:pb, :],
                                         aT[:K, off:off + pb],
                                         ikt[:K, :], start=True, stop=True)
                        iem = st.tile([128, IL], F32, tag="iem", name="iem")
                        nc.vector.tensor_add(iem[:pb], psi[:pb], s_iem[:pb, bt, :])
                        ewi = st.tile([128, IL], F32, tag="ewi", name="ewi")
                        ssi = st.tile([128, 1], F32, name="ssi")
                        nc.scalar.activation(ewi[:pb], iem[:pb], AF.Exp,
                                             scale=1.0, accum_out=ssi[:pb])
                        rsi = st.tile([128, 1], F32, name="rsi")
                        nc.vector.reciprocal(rsi[:pb], ssi[:pb])
                        iwb = st.tile([128, IL], BF16, name="iwb")
                        nc.vector.tensor_scalar_mul(iwb[:pb], ewi[:pb], rsi[:pb])
                        for c in range(2):
                            tr(iwT[:, c, off:off + pb],
                               iwb[:pb, c * 128:(c + 1) * 128], pb, 128)

                    for bt in range(2):
                        pb, off = PBS[bt], BOFF[bt]
                        psqc = psg.tile([128, H], F32, tag="g", name="psqc")
                        if bt == 0:
                            dg = wk.tile([128, L, 128], BF16, tag="diag",
                                         name="dg")
                            hl = L // 2
                            ibh = ident_bf[:pb, :pb]
                            ident_h = bass.AP(tensor=ibh.tensor, offset=ibh.offset,
                                              ap=[ibh.ap[0], [0, hl], ibh.ap[1]])
                            nc.gpsimd.tensor_mul(dg[:pb, :hl, :pb],
                                                 bcast_in(qw_bf[:pb, bt, :hl], pb),
                                                 ident_h)
                            nc.vector.tensor_mul(dg[:pb, hl:, :pb],
                                                 bcast_in(qw_bf[:pb, bt, hl:], pb),
                                                 ident_h)
                            for l in range(L):
                                nc.tensor.matmul(psqc[:pb, :], dg[:pb, l, :pb],
                                                 qv_b0[:pb, l, :],
                                                 start=(l == 0), stop=(l == L - 1))
                        else:
                            qw_pk = st.tile([128, 5], BF16, name="qw_pk")
                            for g4 in range(4):
                                nc.vector.tensor_copy(
                                    qw_pk[32 * g4:32 * (g4 + 1), :],
                                    qw_bf[0:32, 1, g4:L:4])
                            dg1 = wk.tile([128, 5, 32], BF16, tag="dg1",
                                          name="dg1")
                            sid_b = bass.AP(tensor=sid4.tensor,
                                            offset=sid4[:, :].offset,
                                            ap=[sid4[:, :].ap[0], [0, 5],
                                                sid4[:, :].ap[1]])
                            nc.vector.tensor_mul(dg1[:, :, :],
                                                 bcast_in(qw_pk[:, :], 32),
                                                 sid_b)
                            for c in range(5):
                                nc.tensor.matmul(psqc[:pb, :], dg1[:, c, :],
                                                 qv_p1[:, c, :],
                                                 start=(c == 0), stop=(c == 4))
                        qcb = st.tile([128, H], BF16, name="qcb")
                        nc.scalar.copy(qcb[:pb], psqc[:pb])
                        for kt in range(4):
                            tr_add(qcT[:, kt, off:off + pb],
                                   qcb[:pb, kt * 128:(kt + 1) * 128],
                                   s_qvb[:, kt:kt + 1], pb, 128)

                    for ht in range(4):
                        psic = pss.tile([128, BS], F32, tag="s", name="psic")
                        emit_group(psic[:, :],
                                   [(ivv[:, kt, ht * 128:(ht + 1) * 128],
                                     iwT[:, kt, :]) for kt in range(2)])
                        nc.vector.tensor_scalar_add(icT[:, ht, :], psic[:, :],
                                                    s_ivb[:, ht:ht + 1])

                    if t < L:
                        xt = fetch_x(s_aidx, t)
                        xsrc = ColView(xt, 0)
                    else:
                        xsrc = ColView(dec20, 0)
                    for bt in range(2):
                        gru_bt(bt, xsrc, [128, 128, 65], s_gi, s_gh, s_bhh,
                               [(qcT, k, 3 + k) for k in range(4)]
                               + [(icT, k, 7 + k) for k in range(4)],
                               tform=True)

                    for bt in range(2):
                        pb, off = PBS[bt], BOFF[bt]
                        sl = slice(off, off + pb)
                        pso = pss.tile([128, D], F32, tag="s", name="pso")
                        # ctx pairs first: they need no h', so the PE can run
                        # them while the gate-math chain produces hT
                        pairs = [(qcT[:, k, sl], s_out[:, 4 + k, :])
                                 for k in range(4)]
                        pairs += [(icT[:, k, sl], s_out[:, 8 + k, :])
                                  for k in range(4)]
                        pairs.append((ones_bf[:, :pb], s_outb[:]))
                        pairs += [(hT[:, k, sl], s_out[:, k, :]) for k in range(4)]
                        emit_group(pso[:pb, :], pairs)
                        osb = st.tile([128, D], F32, name="osb")
                        nc.scalar.copy(osb[:pb], pso[:pb])
                        nc.sync.dma_start(out_o[t, off:off + pb, :], osb[:pb])
                        if t == MAX_LEN - 2:
                            nc.vector.tensor_copy(o19_sb[bt][:pb], osb[:pb])

                    if t == MAX_LEN - 2:
                        for bt in range(2):
                            pb, off = PBS[bt], BOFF[bt]
                            ob = st.tile([128, D], BF16, name="ob")
                            nc.scalar.copy(ob[:pb], o19_sb[bt][:pb])
                            for kt in range(3):
                                w = 128 if kt < 2 else D - 256
                                tr(o19T[:w, kt, off:off + pb],
                                   ob[:pb, kt * 128:kt * 128 + w], pb, w)
                        for bt in range(2):
                            pb, off = PBS[bt], BOFF[bt]
                            for nci in range(18):
                                ncw = 512 if nci < 17 else V - 17 * 512
                                rhs = wk.tile([128, 3, 512], BF16, tag="lrhs", bufs=4,
                                              name="rhs")
                                for kt in range(3):
                                    nr = 128 if kt < 2 else 65
                                    nc.sync.dma_start(
                                        rhs[:nr, kt, :ncw],
                                        embt_bf[:nr, kt,
                                                nci * 512:nci * 512 + ncw])
                                psl = psg.tile([128, H], F32, tag="g", name="psl")
                                pairs = []
                                for kt in range(3):
                                    nr = 128 if kt < 2 else 65
                                    pairs.append((o19T[:nr, kt, off:off + pb],
                                                  rhs[:nr, kt, :ncw]))
                                emit_group(psl[:pb, :ncw], pairs)
                                nc.scalar.copy(
                                    logit_sb[:pb, nci * 512:nci * 512 + ncw],
                                    psl[:pb, :ncw])
                            if bt == 0:
                                nc.vector.memset(logit_sb[:, V:], -60000.0)
                            mx8 = st.tile([128, 8], BF16, name="mx8")
                            nc.vector.max(mx8[:pb], logit_sb[:pb])
                            ix8 = st.tile([128, 8], U32, name="ix8")
                            nc.vector.max_index(ix8[:pb], mx8[:pb], logit_sb[:pb])
                            scores = st.tile([128, 8], F32, name="scores")
                            for j in range(8):
                                g8 = wk.tile([128, D + 1], F32, tag="gath8",
                                             name="g8")
                                nc.gpsimd.indirect_dma_start(
                                    out=g8[:pb], out_offset=None, in_=emb_aug[:],
                                    in_offset=bass.IndirectOffsetOnAxis(
                                        ap=ix8[:pb, j:j + 1], axis=0))
                                pr = wk.tile([128, D], F32, tag="pr8", name="pr")
                                nc.vector.tensor_mul(pr[:pb], o19_sb[bt][:pb],
                                                     g8[:pb, :D])
                                sj = st.tile([128, 1], F32, name="sj")
                                nc.vector.tensor_reduce(sj[:pb], pr[:pb],
                                                        axis=AX.X, op=ALU.add)
                                nc.vector.tensor_add(scores[:pb, j:j + 1],
                                                     sj[:pb], g8[:pb, D:D + 1])
                            m1 = st.tile([128, 8], F32, name="m1")
                            nc.vector.max(m1[:pb], scores[:pb])
                            j1 = st.tile([128, 8], U32, name="j1")
                            nc.vector.max_index(j1[:pb], m1[:pb], scores[:pb])
                            j1f = st.tile([128, 1], F32, name="j1f")
                            nc.vector.tensor_copy(j1f[:pb], j1[:pb, 0:1])
                            oh = st.tile([128, 8], F32, name="oh")
                            nc.vector.tensor_scalar(out=oh[:pb], in0=iota8[:pb],
                                                    scalar1=j1f[:pb], scalar2=None,
                                                    op0=ALU.is_equal)
                            ix8f = st.tile([128, 8], F32, name="ix8f")
                            nc.vector.tensor_copy(ix8f[:pb], ix8[:pb])
                            nc.vector.tensor_mul(ix8f[:pb], oh[:pb], ix8f[:pb])
                            vsum = st.tile([128, 1], F32, name="vsum")
                            nc.vector.tensor_reduce(vsum[:pb], ix8f[:pb],
                                                    axis=AX.X, op=ALU.add)
                            vidx = st.tile([128, 1], U32, name="vidx")
                            nc.vector.tensor_copy(vidx[:pb], vsum[:pb])
                            gm = wk.tile([128, D], BF16, tag="gath", bufs=6,
                                         name="gm")
                            nc.gpsimd.indirect_dma_start(
                                out=gm[:pb], out_offset=None, in_=emb_bf[:],
                                in_offset=bass.IndirectOffsetOnAxis(
                                    ap=vidx[:pb, 0:1], axis=0))
                            for kt in range(3):
                                w = 128 if kt < 2 else D - 256
                                tr(dec20[:w, kt, off:off + pb],
                                   gm[:pb, kt * 128:kt * 128 + w], pb, w)

    nc.compile()
    return nc


_NC_CACHE = None


def _get_nc():
    global _NC_CACHE
    if _NC_CACHE is None:
        _NC_CACHE = build_nc()
    return _NC_CACHE


def _pad_tiles(a, ntiles):
    rows, cols = a.shape
    out = np.zeros((128 * ntiles, cols), a.dtype)
    out[:rows] = a
    return np.ascontiguousarray(out.reshape(ntiles, 128, cols).transpose(1, 0, 2))


def _prep_shared(inputs):
    bf = np.float16
    f32 = np.float32
    eW = np.asarray(inputs["embed_W"], f32)
    d = {}
    wih = np.asarray(inputs["dec_W_ih"], f32)
    bih = np.asarray(inputs["dec_b_ih"], f32)
    bhh = np.asarray(inputs["dec_b_hh"], f32)
    gi = np.zeros((128 * 11, 3 * H), f32)
    gi[0:D] = wih[:, 0:D].T
    gi[320] = bih + np.concatenate([bhh[:2 * H], np.zeros(H, f32)])
    gi[384:384 + H] = wih[:, D:D + H].T
    gi[896:896 + H] = wih[:, D + H:].T
    gi[:, 0:2 * H] *= 0.5
    gi[:, 2 * H:] *= 2.0
    d["w_gi"] = _pad_tiles(gi.astype(bf), 11)
    gh = np.asarray(inputs["dec_W_hh"], f32).T.copy()
    gh[:, 0:2 * H] *= 0.5
    d["w_gh"] = _pad_tiles(gh.astype(bf), 4)
    d["bhh_n"] = np.ascontiguousarray(bhh[2 * H:].astype(bf)[None, :])
    ewih = np.asarray(inputs["enc_W_ih"], f32)
    ebih = np.asarray(inputs["enc_b_ih"], f32)
    ebhh = np.asarray(inputs["enc_b_hh"], f32)
    egi = np.zeros((128 * 3, 3 * H), f32)
    egi[0:D] = ewih[:, :D].T
    egi[320] = ebih + np.concatenate([ebhh[:2 * H], np.zeros(H, f32)])
    d["w_egi"] = _pad_tiles(egi.astype(bf), 3)
    d["w_egh"] = _pad_tiles(np.asarray(inputs["enc_W_hh"], f32).T.astype(bf), 4)
    d["ebhh_n"] = np.ascontiguousarray(ebhh[2 * H:].astype(bf)[None, :])
    d["w_out"] = _pad_tiles(np.asarray(inputs["out_W"], f32).T.astype(bf), 12)
    d["outb"] = np.ascontiguousarray(
        np.asarray(inputs["out_b"], f32).astype(bf)[None, :])
    d["w_qk"] = _pad_tiles(np.asarray(inputs["qk_W"], f32).T.astype(bf), 4)
    d["qkb"] = np.ascontiguousarray(
        np.asarray(inputs["qk_b"], f32).astype(bf)[None, :])
    d["w_qv"] = _pad_tiles(np.asarray(inputs["qv_W"], f32).T.astype(bf), 4)
    d["qvb_c"] = np.ascontiguousarray(
        np.asarray(inputs["qv_b"], f32).reshape(4, 128).T)
    d["w_ak"] = _pad_tiles(np.asarray(inputs["ak_W"], f32).T.astype(bf), 4)
    d["akb"] = np.ascontiguousarray(
        np.asarray(inputs["ak_b"], f32).astype(bf)[None, :])
    d["w_ik"] = _pad_tiles(np.asarray(inputs["ik_W"], f32).T.astype(bf), 2)
    ikb = np.zeros((128, 1), f32)
    ikb[:K, 0] = np.asarray(inputs["ik_b"], f32)
    d["ikb_c"] = ikb
    d["w_iv"] = _pad_tiles(np.asarray(inputs["iv_W"], f32).T.astype(bf), 2)
    d["ivb_c"] = np.ascontiguousarray(
        np.asarray(inputs["iv_b"], f32).reshape(4, 128).T)
    d["emb_bf"] = eW.astype(bf)
    wd_b = np.asarray(inputs["wd_b"], f32)
    d["emb_aug"] = np.ascontiguousarray(np.concatenate([eW, wd_b[:, None]], 1))
    aug = np.zeros((128 * 3, VP), f32)
    aug[:D, :V] = eW.T
    aug[320, :V] = wd_b
    d["embt_bf"] = _pad_tiles(aug.astype(bf), 3)
    return d


def _idx_cols(seq_rows):
    out = np.zeros((128, 2 * L), np.uint32)
    for t in range(L):
        out[:, 2 * t] = seq_rows[0:128, t]
        out[:32, 2 * t + 1] = seq_rows[128:160, t]
    return out


def _build_maps(inputs, shared):
    f32 = np.float32
    bf = np.float16
    ques = np.asarray(inputs["ques_seqs"]).astype(np.uint32)
    ans = np.asarray(inputs["ans_seqs"]).astype(np.uint32)
    qlens = np.asarray(inputs["ques_lens"]).astype(np.int64)
    img = np.asarray(inputs["img_seqs"], f32)
    maps = []
    for s in range(NCORES):
        m = dict(shared)
        r0 = s * BS
        m["q_idx"] = _idx_cols(ques[r0:r0 + BS, :L])
        m["a_idx"] = _idx_cols(ans[r0:r0 + BS, :L])
        qm = np.full((128, 2, L), NEG, f32)
        lens = qlens[r0:r0 + BS]
        for bt, (pb, off) in enumerate(zip(PBS, BOFF)):
            for b in range(pb):
                qm[b, bt, :lens[off + b]] = 0.0
        m["qe_mask"] = qm
        im = np.full((128, 2, IL), NEG, f32)
        for bt, (pb, off) in enumerate(zip(PBS, BOFF)):
            for b in range(pb):
                gimg = (off + b) // ROUNDS
                im[b, bt, gimg * 16:(gimg + 1) * 16] = 0.0
        m["ie_mask"] = im
        imgs = img[s * 16:(s + 1) * 16].reshape(IL, 256)
        it = np.zeros((128 * 2, IL), f32)
        it[:256] = imgs.T
        m["img_t"] = np.ascontiguousarray(
            it.reshape(2, 128, IL).transpose(1, 0, 2)).astype(bf)
        maps.append(m)
    return maps


def kernel(**inputs):
    nc = _get_nc()
    shared = _prep_shared(inputs)
    in_maps = _build_maps(inputs, shared)
    from concourse.bass_utils import run_bass_kernel_spmd
    res = run_bass_kernel_spmd(nc, in_maps, core_ids=list(range(NCORES)))
    outs = []
    for s in range(NCORES):
        o = np.asarray(res.results[s]["out_o"])
        outs.append(np.ascontiguousarray(o.transpose(1, 0, 2)))
    return np.concatenate(outs, 0).astype(np.float32)

